# revision 12
# baseline (speedup 1.0000x reference)
"""Trainium2 Bass kernel for nn_LiquidNeuralNetwork (B=512, S=1024, IN=16, HID=64).

Strategy
--------
The reference integrates dh/dt = (-h + tanh(h) @ W_hh.T + inp + bias) / tau
with RK4 x 4 substeps per timestep (16 sequential tanh+matmul rounds per
step).  At dt = 1/1023 the integration error of far cheaper schemes is orders
of magnitude below f32 rounding noise, so we integrate the same ODE with an
exponential integrator + AB2 extrapolation of the (tiny) tanh coupling term:

    H_s = a*H_{s-1} + b*(c_s + 1.5*g_{s-1} - 0.5*g_{s-2}),
    g_s = W_hh @ tanh(H_s),  a = exp(-dt/tau), b = 1 - a,
    c_s = W_ih @ (W_in x_s + b_in) + bias   (precomputed, hidden-major)

which agrees with the reference to ~6e-6 (the f32 noise floor of the
reference itself) while needing ONE tanh + matmul round per timestep.

On-device layout: hidden on partitions, batch on free dim; batch sharded
8 ways (64 per core).  Per round the PSUM bank accumulates the full affine
update via matmuls only:

    bank_r[0:64]  = diag(b) @ c_r + diag(a) @ hm_r + [Wp;Wm] @ [th_r;th_{r-1}]
    bank_r[64]    = W_out @ th_r          (the per-step scalar output)

with Wp = (1.5*b*W_hh)^T, Wm = (-0.5*b*W_hh)^T.  tanh runs on ACT straight
from PSUM; DVE copies bank->SBUF (h materialization + output-row collection);
everything except ACT->PE->ACT is off the critical path.
"""

import os
import numpy as np

import concourse.bacc as bacc
import concourse.tile as tile
from concourse import mybir
from concourse.bass_utils import run_bass_kernel_spmd

F32 = mybir.dt.float32
H = 64          # hidden
BIN = 16        # input features
B_FULL = 512
S = int(os.environ.get("LNN_S", "1024"))   # harness always runs 1024
N_CORES = 8
B = B_FULL // N_CORES   # 64 per-core batch
SEG = 128 if S % 128 == 0 else S           # output segment length (steps)
N_SEG = S // SEG

TRACE = bool(int(os.environ.get("LNN_TRACE", "0")))
SCHEME = os.environ.get("LNN_SCHEME", "v7")   # "v7" | "pair" | "pairz" | "e2"

GRP = 8                       # pair-slots per bulk output matmul (v7)
NGRP = (S // 2) // GRP

NPAIR = S // 2                 # pair rounds
SEGP = NPAIR if NPAIR <= 256 else 256   # pair-slots per output segment
NSEGP = NPAIR // SEGP

_cached = {}


def _build_program():
    """Build + compile the Bass program (same NEFF for all cores)."""
    nc = bacc.Bacc("TRN2", target_bir_lowering=False, debug=False)

    in_C = nc.dram_tensor("in_C", (S, H, B), F32, kind="ExternalInput").ap()
    in_Aev = nc.dram_tensor("in_Aev", (2 * H, H + 1), F32, kind="ExternalInput").ap()
    in_Aod = nc.dram_tensor("in_Aod", (2 * H, H + 1), F32, kind="ExternalInput").ap()
    in_Atl = nc.dram_tensor("in_Atl", (2 * H, H + 1), F32, kind="ExternalInput").ap()
    in_Db = nc.dram_tensor("in_Db", (H, H + 1), F32, kind="ExternalInput").ap()
    in_Da = nc.dram_tensor("in_Da", (H, H), F32, kind="ExternalInput").ap()
    out_dram = nc.dram_tensor("out", (N_SEG, SEG * B), F32, kind="ExternalOutput").ap()

    TANH = mybir.ActivationFunctionType.Tanh

    with tile.TileContext(nc) as tc:
        with (
            tc.tile_pool(name="wts", bufs=1) as wts,
            tc.tile_pool(name="thp", bufs=1) as thp,
            tc.tile_pool(name="osb", bufs=2) as osbp,
            tc.tile_pool(name="cp", bufs=10) as cp,
            tc.tile_pool(name="hmp", bufs=3) as hmp,
            tc.tile_pool(name="hbank", bufs=4, space="PSUM") as hbank,
        ):
            t_Aev = wts.tile([2 * H, H + 1], F32, tag="aev")
            t_Aod = wts.tile([2 * H, H + 1], F32, tag="aod")
            t_Atl = wts.tile([2 * H, H + 1], F32, tag="atl")
            t_Db = wts.tile([H, H + 1], F32, tag="db")
            t_Da = wts.tile([H, H], F32, tag="da")
            nc.sync.dma_start(out=t_Aev, in_=in_Aev)
            nc.sync.dma_start(out=t_Aod, in_=in_Aod)
            nc.sync.dma_start(out=t_Atl, in_=in_Atl)
            nc.sync.dma_start(out=t_Db, in_=in_Db)
            nc.sync.dma_start(out=t_Da, in_=in_Da)

            # persistent tanh tile: half0 = th of even rounds, half1 = odd
            t_th = thp.tile([2 * H, B], F32, tag="th")
            nc.vector.memset(t_th, 0.0)

            # output staging: only partition 64 is used; slot o at free
            # offset (o % SEG)*B.  Two tiles ping-pong across segments.
            t_osb = [osbp.tile([H + 1, SEG * B], F32, tag="osb", name=f"t_osb{i}")
                     for i in range(2)]

            prev_bank = None
            for r in range(1, S):
                t_c = cp.tile([H, B], F32, tag="c")
                nc.sync.dma_start(out=t_c, in_=in_C[r])

                bank = hbank.tile([H + 1, B], F32, tag="bank")
                last = r == 1
                # M4 first (start=True): clears rows 0..64 (col H of Db is 0)
                nc.tensor.matmul(bank, t_Db, t_c, start=True, stop=last)

                if r >= 2:
                    o = r - 2          # output index evacuated this round
                    seg, slot = divmod(o, SEG)
                    # evacuate prev bank's output row (lane-aligned copy)
                    nc.vector.tensor_copy(
                        t_osb[seg % 2][H:H + 1, slot * B:(slot + 1) * B],
                        prev_bank[H:H + 1, :],
                    )
                    if slot == SEG - 1:
                        nc.sync.dma_start(
                            out=out_dram[seg],
                            in_=t_osb[seg % 2][H:H + 1, :],
                        )
                    # h materialization for the decay term
                    t_hm = hmp.tile([H, B], F32, tag="hm")
                    nc.vector.tensor_copy(t_hm, prev_bank[:H, :])
                    # tanh straight from PSUM into this round's th half
                    half = r % 2
                    nc.scalar.activation(
                        t_th[half * H:(half + 1) * H, :], prev_bank[:H, :], TANH)
                    nc.tensor.matmul(bank[:H, :], t_Da, t_hm,
                                     start=False, stop=False)
                    t_A = t_Aev if r % 2 == 0 else t_Aod
                    nc.tensor.matmul(bank, t_A, t_th, start=False, stop=True)
                prev_bank = bank

            # tail: evacuate out_{S-2}; th_S = tanh(H_{S-1}); out_{S-1}
            o = S - 2
            seg, slot = divmod(o, SEG)
            nc.vector.tensor_copy(
                t_osb[seg % 2][H:H + 1, slot * B:(slot + 1) * B],
                prev_bank[H:H + 1, :],
            )
            half = S % 2
            nc.scalar.activation(
                t_th[half * H:(half + 1) * H, :], prev_bank[:H, :], TANH)
            tbank = hbank.tile([H + 1, B], F32, tag="bank")
            nc.tensor.matmul(tbank, t_Atl, t_th, start=True, stop=True)
            o = S - 1
            seg, slot = divmod(o, SEG)
            nc.vector.tensor_copy(
                t_osb[seg % 2][H:H + 1, slot * B:(slot + 1) * B],
                tbank[H:H + 1, :],
            )
            nc.sync.dma_start(out=out_dram[seg], in_=t_osb[seg % 2][H:H + 1, :])

    nc.compile()
    return nc


def _build_program_v7():
    """Pair scheme v7: no f32 matmuls, one bf16 tanh/round, bulk output.

    State per round r (2 timesteps): P = [H_s; H_{s+1}] and the prescaled
    decay copy Q = [a*H_{s+1}; a^2*H_{s+1}], both f32 PSUM [2H, B].

        P_r = Q_{r-1} + Cb_r + LT1 @ T1_r        (DVE base + 1 bf16 matmul)
        Q_r = a^2*Q_{r-1} + C2_r + LT12 @ T1_r   (DVE mul+add + 1 bf16 matmul)
        T1_r = tanh(P_{r-1})  (single bf16 ACT, written into a group buffer)

    Cb/C2 are host-premixed bf16 tiles DMA'd on the Sync and ACT hwdge
    queues respectively.  Outputs: T1 tiles accumulate in a [2H, GRP*B]
    group buffer; every GRP rounds one bulk matmul LO @ thbuf produces
    [2, GRP*B] in PSUM, evacuated by GpSimd and DMA'd out via swdge.
    """
    nc = bacc.Bacc("TRN2", target_bir_lowering=False, debug=False)

    BF16 = mybir.dt.bfloat16
    NP = NPAIR

    NCHUNK = NP // GRP
    in_Cb = nc.dram_tensor("in_Cb", (NCHUNK, 2 * H, GRP * B), BF16,
                           kind="ExternalInput").ap()
    in_C2 = nc.dram_tensor("in_C2", (NCHUNK, 2 * H, GRP * B), BF16,
                           kind="ExternalInput").ap()
    in_LT1 = nc.dram_tensor("in_LT1", (2 * H, 2 * H), BF16,
                            kind="ExternalInput").ap()
    in_LT12 = nc.dram_tensor("in_LT12", (2 * H, 2 * H), BF16,
                             kind="ExternalInput").ap()
    in_LO = nc.dram_tensor("in_LO", (2 * H, 2), BF16,
                           kind="ExternalInput").ap()
    in_a2 = nc.dram_tensor("in_a2", (2 * H, 1), F32, kind="ExternalInput").ap()
    out_dram = nc.dram_tensor("out", (NGRP, 2, GRP * B), F32,
                              kind="ExternalOutput").ap()

    TANH = mybir.ActivationFunctionType.Tanh

    with tile.TileContext(nc) as tc:
        with (
            tc.tile_pool(name="wts", bufs=1) as wts,
            tc.tile_pool(name="cbp", bufs=4) as cbp,
            tc.tile_pool(name="c2p", bufs=4) as c2p,
            tc.tile_pool(name="tmpp", bufs=3) as tmpp,
            tc.tile_pool(name="thb", bufs=2) as thbp,
            tc.tile_pool(name="osb", bufs=2) as osbp,
            tc.tile_pool(name="pP", bufs=3, space="PSUM") as pP,
            tc.tile_pool(name="pQ", bufs=3, space="PSUM") as pQ,
            tc.tile_pool(name="pO", bufs=2, space="PSUM") as pO,
        ):
            t_LT1 = wts.tile([2 * H, 2 * H], BF16, name="t_LT1")
            t_LT12 = wts.tile([2 * H, 2 * H], BF16, name="t_LT12")
            t_LO = wts.tile([2 * H, 2], BF16, name="t_LO")
            t_a2 = wts.tile([2 * H, 1], F32, name="t_a2")
            nc.sync.dma_start(out=t_LT1, in_=in_LT1)
            nc.sync.dma_start(out=t_LT12, in_=in_LT12)
            nc.sync.dma_start(out=t_LO, in_=in_LO)
            nc.sync.dma_start(out=t_a2, in_=in_a2)

            t_thb = [thbp.tile([2 * H, GRP * B], BF16, tag="thb",
                               name=f"t_thb{i}") for i in range(2)]

            # chunked c-tile DMA: one [2H, GRP*B] transfer per GRP rounds
            # per stream, both on the Sync hwdge queue; prefetch 2 chunks.
            cb_t, c2_t = {}, {}

            def fetch(k):
                if k >= NCHUNK:
                    return
                cb_t[k] = cbp.tile([2 * H, GRP * B], BF16, tag="cb",
                                   name=f"cb{k % 4}")
                nc.sync.dma_start(out=cb_t[k], in_=in_Cb[k])
                c2_t[k] = c2p.tile([2 * H, GRP * B], BF16, tag="c2",
                                   name=f"c2{k % 4}")
                nc.sync.dma_start(out=c2_t[k], in_=in_C2[k])

            for k in range(3):
                fetch(k)

            # boot: P0 = Cb[0], Q0 = C2[0]  (H_0 = 0)
            P = pP.tile([2 * H, B], F32, tag="P")
            Q = pQ.tile([2 * H, B], F32, tag="Q")
            nc.vector.tensor_copy(P, cb_t[0][:, 0:B])
            nc.vector.tensor_copy(Q, c2_t[0][:, 0:B])

            prev_P, prev_Q = P, Q
            for r in range(1, NP):
                g, slot = divmod(r - 1, GRP)
                ck, cs = divmod(r, GRP)
                t_cb = cb_t[ck][:, cs * B:(cs + 1) * B]
                t_c2 = c2_t[ck][:, cs * B:(cs + 1) * B]

                # single bf16 tanh straight from PSUM into the group buffer
                T1 = t_thb[g % 2][:, slot * B:(slot + 1) * B]
                nc.scalar.activation(T1, prev_P, TANH)
                if cs == 1:
                    fetch(ck + 2)

                P = pP.tile([2 * H, B], F32, tag="P")
                Q = pQ.tile([2 * H, B], F32, tag="Q")
                # P base on DVE; Q decay scale on ACT, Q base add on DVE
                nc.vector.tensor_add(P, prev_Q, t_cb)
                t_tmp = tmpp.tile([2 * H, B], F32, tag="tmp")
                nc.scalar.mul(t_tmp, prev_Q, t_a2)
                nc.vector.tensor_add(Q, t_tmp, t_c2)

                # coupling matmuls accumulate onto the DVE-written bases
                nc.tensor.matmul(P, t_LT1, T1, start=False, stop=True,
                                 skip_group_check=True)
                nc.tensor.matmul(Q, t_LT12, T1, start=False, stop=True,
                                 skip_group_check=True)

                if slot == GRP - 1:
                    ob = pO.tile([2, GRP * B], F32, tag="ob")
                    nc.tensor.matmul(ob, t_LO, t_thb[g % 2],
                                     start=True, stop=True)
                    t_os = osbp.tile([2, GRP * B], F32, tag="os")
                    nc.vector.tensor_copy(t_os, ob)
                    nc.gpsimd.dma_start(out=out_dram[g], in_=t_os)

                prev_P, prev_Q = P, Q

            # tail: T1(NP) completes the last group
            g, slot = NGRP - 1, GRP - 1
            T1 = t_thb[g % 2][:, slot * B:(slot + 1) * B]
            nc.scalar.activation(T1, prev_P, TANH)
            ob = pO.tile([2, GRP * B], F32, tag="ob")
            nc.tensor.matmul(ob, t_LO, t_thb[g % 2], start=True, stop=True)
            t_os = osbp.tile([2, GRP * B], F32, tag="os")
            nc.vector.tensor_copy(t_os, ob)
            nc.gpsimd.dma_start(out=out_dram[g], in_=t_os)

    nc.compile()
    return nc


def _v7_weights(a, b, W_hh, W_out):
    """LT1/LT12 as in pairz, plus bf16 LO and the a^2 decay vector."""
    import ml_dtypes
    wm = _pairz_weights(a, b, W_hh, W_out)
    return {
        "in_LT1": wm["in_LT1"],
        "in_LT12": wm["in_LT12"],
        "in_LO": wm["in_LO"].astype(ml_dtypes.bfloat16),
        "in_a2": wm["in_a2"],
    }


def _v7_cc(Cc, a, b):
    """Host Cb/C2 tiles [NPAIR, 2H, B] bf16 for one core (see _pairz_cc)."""
    import ml_dtypes
    ab = a * b
    ce = Cc[:, 0::2, :].astype(np.float64)    # c_{2r}   [B, NPAIR, H]
    co = Cc[:, 1::2, :].astype(np.float64)    # c_{2r+1}
    z = ab * ce + b * co
    Bc = Cc.shape[0]
    Cb = np.empty((NPAIR, 2 * H, Bc), np.float32)
    C2 = np.empty((NPAIR, 2 * H, Bc), np.float32)
    Cb[:, :H] = (b * ce).transpose(1, 2, 0)
    Cb[:, H:] = z.transpose(1, 2, 0)
    C2[:, :H] = (a * z).transpose(1, 2, 0)
    C2[:, H:] = (a * a * z).transpose(1, 2, 0)
    c1 = co[:, 0, :]                          # [B, H]
    Cb[0, :H] = 0.0
    Cb[0, H:] = (b * c1).T
    C2[0, :H] = (ab * c1).T
    C2[0, H:] = (a * ab * c1).T

    def chunk(arr):
        return np.ascontiguousarray(
            arr.reshape(NPAIR // GRP, GRP, 2 * H, Bc)
            .transpose(0, 2, 1, 3)
            .reshape(NPAIR // GRP, 2 * H, GRP * Bc)
        ).astype(ml_dtypes.bfloat16)

    return chunk(Cb), chunk(C2)


def _build_program_pairz():
    """Pair scheme v6 ("zlite"): 2 steps per round, NO f32 matmuls on PE.

    One PSUM tile P_r [128,128] per round: cols 0:64 "bank" = [H_s; H_{s+1}],
    cols 64:128 "bank2" = [a*H_{s+1}; a^2*H_{s+1}] (pre-scaled decay copies,
    maintained so the next round's injections are lane-aligned DVE ops):

        bank_r   = bank2_{r-1} + Cb''_r + coupling(th)          (1 DVE add)
        bank2_r  = a^2*bank2_{r-1} + C2''_r + coupling2(th)     (mul + add)

    with all c-terms host-premixed into C''.  PE does only: 2 bf16 coupling
    matmuls (accumulating onto the DVE-written base via start=False) and the
    f32 output matvec.  tanh pair: bf16 (coupling) + f32 (out matvec).
    """
    nc = bacc.Bacc("TRN2", target_bir_lowering=False, debug=False)

    BF16 = mybir.dt.bfloat16

    in_C = nc.dram_tensor("in_C", (NPAIR, 2 * H, 2 * H), F32,
                          kind="ExternalInput").ap()
    ins = {}
    for nm in ("LT1", "LT12"):
        ins[nm] = nc.dram_tensor(f"in_{nm}", (2 * H, 2 * H), BF16,
                                 kind="ExternalInput").ap()
    ins["LO"] = nc.dram_tensor("in_LO", (2 * H, 2), F32,
                               kind="ExternalInput").ap()
    in_a2 = nc.dram_tensor("in_a2", (2 * H, 1), F32, kind="ExternalInput").ap()
    out_dram = nc.dram_tensor("out", (NSEGP, 2, SEGP * B), F32,
                              kind="ExternalOutput").ap()

    TANH = mybir.ActivationFunctionType.Tanh

    with tile.TileContext(nc) as tc:
        with (
            tc.tile_pool(name="wts", bufs=1) as wts,
            tc.tile_pool(name="thp", bufs=3) as thp,
            tc.tile_pool(name="thf", bufs=3) as thfp,
            tc.tile_pool(name="osb", bufs=2) as osbp,
            tc.tile_pool(name="ccp", bufs=6) as ccp,
            tc.tile_pool(name="tmpp", bufs=3) as tmpp,
            tc.tile_pool(name="pbank", bufs=4, space="PSUM") as pbank,
            tc.tile_pool(name="obank", bufs=3, space="PSUM") as obankp,
        ):
            t_w = {}
            for nm in ("LT1", "LT12"):
                t_w[nm] = wts.tile([2 * H, 2 * H], BF16, name=f"t_{nm}")
                nc.sync.dma_start(out=t_w[nm], in_=ins[nm])
            t_w["LO"] = wts.tile([2 * H, 2], F32, name="t_LO")
            nc.sync.dma_start(out=t_w["LO"], in_=ins["LO"])
            t_a2 = wts.tile([2 * H, 1], F32, name="t_a2")
            nc.sync.dma_start(out=t_a2, in_=in_a2)

            t_osb = [osbp.tile([2, SEGP * B], F32, tag="osb", name=f"t_osb{i}")
                     for i in range(2)]

            # boot: P_0 = C''_0 (H_0 = 0 so no decay/coupling terms)
            t_cc = ccp.tile([2 * H, 2 * H], F32, tag="cc")
            nc.sync.dma_start(out=t_cc, in_=in_C[0])
            P = pbank.tile([2 * H, 2 * H], F32, tag="P")
            nc.vector.tensor_copy(P, t_cc)

            prev_P = P
            prev_thf = None
            pending = []

            def flush_one():
                ob_t, m = pending.pop(0)
                seg, slot = divmod(m, SEGP)
                nc.vector.tensor_copy(
                    t_osb[seg % 2][0:2, slot * B:(slot + 1) * B], ob_t)
                if slot == SEGP - 1:
                    nc.sync.dma_start(out=out_dram[seg],
                                      in_=t_osb[seg % 2][0:2, :])

            for r in range(1, NPAIR):
                t_cc = ccp.tile([2 * H, 2 * H], F32, tag="cc")
                nc.sync.dma_start(out=t_cc, in_=in_C[r])

                P = pbank.tile([2 * H, 2 * H], F32, tag="P")
                # critical-path injection: bank base = bank2_prev + Cb''
                nc.vector.tensor_add(P[:, :2 * H - H], prev_P[:, H:H + H],
                                     t_cc[:, 0:H])
                # off-path: bank2 base = a^2*bank2_prev + C2''
                t_tmp = tmpp.tile([2 * H, H], F32, tag="tmp")
                nc.vector.tensor_scalar_mul(t_tmp, prev_P[:, H:H + H], t_a2)
                nc.vector.tensor_add(P[:, H:H + H], t_tmp, t_cc[:, H:H + H])

                # tanh pair from prev bank
                T1 = thp.tile([2 * H, B], BF16, tag="t1")
                nc.scalar.activation(T1, prev_P[:, 0:H], TANH)
                t_thf = thfp.tile([2 * H, B], F32, tag="thf")
                nc.scalar.activation(t_thf, prev_P[:, 0:H], TANH)

                # previous round's out matvec (f32) while ACT runs
                if prev_thf is not None:
                    ob = obankp.tile([2, B], F32, tag="ob")
                    nc.tensor.matmul(ob, t_w["LO"], prev_thf,
                                     start=True, stop=True)
                    pending.append((ob, r - 2))
                if len(pending) > 1:
                    flush_one()

                # coupling matmuls accumulate onto the DVE-written base
                nc.tensor.matmul(P[:, 0:H], t_w["LT1"], T1,
                                 start=False, stop=False,
                                 skip_group_check=True)
                nc.tensor.matmul(P[:, H:H + H], t_w["LT12"], T1,
                                 start=False, stop=True,
                                 skip_group_check=True)

                prev_P, prev_thf = P, t_thf

            # tail
            ob = obankp.tile([2, B], F32, tag="ob")
            nc.tensor.matmul(ob, t_w["LO"], prev_thf, start=True, stop=True)
            pending.append((ob, NPAIR - 2))
            t_thf = thfp.tile([2 * H, B], F32, tag="thf")
            nc.scalar.activation(t_thf, prev_P[:, 0:H], TANH)
            ob = obankp.tile([2, B], F32, tag="ob")
            nc.tensor.matmul(ob, t_w["LO"], t_thf, start=True, stop=True)
            pending.append((ob, NPAIR - 1))
            while pending:
                flush_one()

    nc.compile()
    return nc


def _pairz_weights(a, b, W_hh, W_out):
    """Host lhsT matrices + per-partition a^2 vector for the v6 scheme."""
    import ml_dtypes
    W = W_hh.astype(np.float64)
    wout = W_out[0].astype(np.float64)
    ab = a * b

    def blk(v):
        return (v[:, None] * W).T

    cp1, cm1 = 1.5 * b, -0.5 * b
    cp2, cm2 = 1.5 * ab + 2.5 * b, -0.5 * ab - 1.5 * b

    LT1 = np.zeros((2 * H, 2 * H))
    LT1[:H, :H] = blk(cm1)
    LT1[H:, :H] = blk(cp1)
    LT1[:H, H:] = blk(cm2)
    LT1[H:, H:] = blk(cp2)
    LT12 = np.zeros((2 * H, 2 * H))
    LT12[:H, :H] = blk(a * cm2)
    LT12[H:, :H] = blk(a * cp2)
    LT12[:H, H:] = blk(a * a * cm2)
    LT12[H:, H:] = blk(a * a * cp2)
    LO = np.zeros((2 * H, 2))
    LO[:H, 0] = wout
    LO[H:, 1] = wout
    a2v = np.concatenate([a * a, a * a]).reshape(2 * H, 1)
    return {"in_LT1": LT1.astype(ml_dtypes.bfloat16),
            "in_LT12": LT12.astype(ml_dtypes.bfloat16),
            "in_LO": LO.astype(np.float32),
            "in_a2": a2v.astype(np.float32)}


def _pairz_cc(Cc, a, b):
    """Host C'' quadrant tiles [NPAIR, 2H, 2H] for one core.

    Cc: [B, S, H] raw input-current.  Quadrants (rows x cols):
      [:, :H]  (bank col):  [b*c_s ; ab*c_s + b*c_{s+1}]
      [:, H:]  (bank2 col): [a^2 b*c_s + ab*c_{s+1} ; a^3 b*c_s + a^2 b*c_{s+1}]
    Boot tile (r=0, H_0=0): bank col = [0 ; b*c_1],
      bank2 col = [ab*c_1 ; a^2 b*c_1].
    """
    ab = a * b
    ce = Cc[:, 0::2, :].astype(np.float64)    # c_{2r}   [B, NPAIR, H]
    co = Cc[:, 1::2, :].astype(np.float64)    # c_{2r+1}
    out = np.empty((NPAIR, 2 * H, 2 * H), np.float32)
    # bank col
    out[:, :H, :H] = (b * ce).transpose(1, 2, 0)
    out[:, H:, :H] = (ab * ce + b * co).transpose(1, 2, 0)
    # bank2 col
    out[:, :H, H:] = (a * (ab * ce + b * co)).transpose(1, 2, 0)
    out[:, H:, H:] = (a * a * (ab * ce + b * co)).transpose(1, 2, 0)
    # boot overrides (c_0 unused, H_0 = 0)
    c1 = co[:, 0, :]                          # [B, H]
    out[0, :H, :H] = 0.0
    out[0, H:, :H] = (b * c1).T
    out[0, :H, H:] = (ab * c1).T
    out[0, H:, H:] = (a * ab * c1).T
    return out


def _build_program_pair():
    """Pair-corrected scheme v2: 2 timesteps per tanh round (S/2 rounds).

    PSUM bank halves = [H_s ; H_{s+1}^pred]; one bf16 ACT tanh covers both
    and feeds the (tiny) tanh-coupling matmuls LT1/LT2 in bf16; a second f32
    tanh feeds the f32 output matvec.  The c-injection is folded into the
    f32 decay matmul LH via a host-prescaled C'' tile DMA'd into the hm
    tile, whose lower half gets H_{s-1} added by one DVE op:
        hm = [b*c_{s+1} ; (b/a)*c_s + H_{s-1}]
        LH @ hm = [a*H_{s-1}+b*c_s ; a^2*H_{s-1}+ab*c_s+b*c_{s+1}]
    """
    nc = bacc.Bacc("TRN2", target_bir_lowering=False, debug=False)

    BF16 = mybir.dt.bfloat16
    GDT = BF16 if os.environ.get("LNN_GDT", "bf16") == "bf16" else F32

    in_C = nc.dram_tensor("in_C", (NPAIR, 2 * H, B), F32,
                          kind="ExternalInput").ap()
    ins = {}
    for nm in ("LH", "LB"):
        ins[nm] = nc.dram_tensor(f"in_{nm}", (2 * H, 2 * H), F32,
                                 kind="ExternalInput").ap()
    for nm in ("LT1", "LT2"):
        ins[nm] = nc.dram_tensor(f"in_{nm}", (2 * H, 2 * H), GDT,
                                 kind="ExternalInput").ap()
    ins["LO"] = nc.dram_tensor("in_LO", (2 * H, 2), F32,
                               kind="ExternalInput").ap()
    out_dram = nc.dram_tensor("out", (NSEGP, 2, SEGP * B), F32,
                              kind="ExternalOutput").ap()

    TANH = mybir.ActivationFunctionType.Tanh

    with tile.TileContext(nc) as tc:
        with (
            tc.tile_pool(name="wts", bufs=1) as wts,
            tc.tile_pool(name="thp", bufs=4) as thp,
            tc.tile_pool(name="thf", bufs=3) as thfp,
            tc.tile_pool(name="thz", bufs=1) as thz,
            tc.tile_pool(name="osb", bufs=2) as osbp,
            tc.tile_pool(name="hmp", bufs=8) as hmp,
            tc.tile_pool(name="hbank", bufs=4, space="PSUM") as hbank,
            tc.tile_pool(name="obank", bufs=3, space="PSUM") as obankp,
        ):
            t_w = {}
            for nm in ("LH", "LB"):
                t_w[nm] = wts.tile([2 * H, 2 * H], F32, name=f"t_{nm}")
                nc.sync.dma_start(out=t_w[nm], in_=ins[nm])
            for nm in ("LT1", "LT2"):
                t_w[nm] = wts.tile([2 * H, 2 * H], GDT, name=f"t_{nm}")
                nc.sync.dma_start(out=t_w[nm], in_=ins[nm])
            t_w["LO"] = wts.tile([2 * H, 2], F32, name="t_LO")
            nc.sync.dma_start(out=t_w["LO"], in_=ins["LO"])

            t_zero = thz.tile([2 * H, B], GDT, tag="t1zero")
            nc.vector.memset(t_zero, 0.0)
            t_osb = [osbp.tile([2, SEGP * B], F32, tag="osb", name=f"t_osb{i}")
                     for i in range(2)]

            # boot: bank_0 = [0 ; b*c_1]  (C''_0 half0 = b*c_1)
            t_hm = hmp.tile([2 * H, B], F32, tag="hm")
            nc.sync.dma_start(out=t_hm, in_=in_C[0])
            bank = hbank.tile([2 * H, B], F32, tag="bank")
            nc.tensor.matmul(bank, t_w["LB"], t_hm, start=True, stop=True)

            prev_bank = bank
            prev_T1 = t_zero
            prev_thf = None           # f32 tanh pair awaiting its out matvec
            pending = []              # [(ob_tile, slot_index)] not yet evac'd

            def flush_one():
                ob_t, m = pending.pop(0)
                seg, slot = divmod(m, SEGP)
                nc.vector.tensor_copy(
                    t_osb[seg % 2][0:2, slot * B:(slot + 1) * B], ob_t)
                if slot == SEGP - 1:
                    nc.sync.dma_start(out=out_dram[seg],
                                      in_=t_osb[seg % 2][0:2, :])

            for r in range(1, NPAIR):
                t_hm = hmp.tile([2 * H, B], F32, tag="hm")
                nc.sync.dma_start(out=t_hm, in_=in_C[r])

                bank = hbank.tile([2 * H, B], F32, tag="bank")
                # bf16 matmul first (FWL-friendly after last round's bf16 LT1)
                nc.tensor.matmul(bank, t_w["LT2"], prev_T1,
                                 start=True, stop=False)
                # the two f32 matmuls adjacent: previous round's out matvec,
                # then the decay+input injection
                if prev_thf is not None:
                    ob = obankp.tile([2, B], F32, tag="ob")
                    nc.tensor.matmul(ob, t_w["LO"], prev_thf,
                                     start=True, stop=True)
                    pending.append((ob, r - 2))

                # tanh pair: bf16 for the coupling path (critical), f32 for
                # the output matvec (off critical path)
                T1 = thp.tile([2 * H, B], GDT, tag="t1")
                nc.scalar.activation(T1, prev_bank, TANH)
                t_thf = thfp.tile([2 * H, B], F32, tag="thf")
                nc.scalar.activation(t_thf, prev_bank, TANH)

                # hm lower half += H_{s-1} (from prev bank)
                nc.vector.tensor_add(t_hm[H:, :], t_hm[H:, :],
                                     prev_bank[H:, :])

                if len(pending) > 1:
                    flush_one()

                nc.tensor.matmul(bank, t_w["LH"], t_hm, start=False,
                                 stop=False)
                nc.tensor.matmul(bank, t_w["LT1"], T1, start=False, stop=True)

                prev_bank, prev_T1, prev_thf = bank, T1, t_thf

            # tail: emit out matvecs for the last two tanh pairs, flush all
            ob = obankp.tile([2, B], F32, tag="ob")
            nc.tensor.matmul(ob, t_w["LO"], prev_thf, start=True, stop=True)
            pending.append((ob, NPAIR - 2))
            t_thf = thfp.tile([2 * H, B], F32, tag="thf")
            nc.scalar.activation(t_thf, prev_bank, TANH)
            ob = obankp.tile([2, B], F32, tag="ob")
            nc.tensor.matmul(ob, t_w["LO"], t_thf, start=True, stop=True)
            pending.append((ob, NPAIR - 1))
            while pending:
                flush_one()   # final segment's DMA fires on its last slot

    nc.compile()
    return nc



def _pair_weights(a, b, W_hh, W_out):
    """Host lhsT matrices for the pair-corrected scheme (f64 in)."""
    import ml_dtypes
    gdt = (ml_dtypes.bfloat16 if os.environ.get("LNN_GDT", "bf16") == "bf16"
           else np.float32)
    W = W_hh.astype(np.float64)
    wout = W_out[0].astype(np.float64)
    ab, a2, a2b = a * b, a * a, a * a * b

    def blk(v):
        return (v[:, None] * W).T

    LH = np.zeros((2 * H, 2 * H))
    LH[:H, H:] = np.eye(H)
    LH[H:, :H] = np.diag(a)
    LH[H:, H:] = np.diag(a2)
    LT1 = np.zeros((2 * H, 2 * H))
    LT1[:H, :H] = blk(-0.5 * b + 1.5 * ab)
    LT1[:H, H:] = blk(-0.5 * ab + 1.5 * a2b - 1.5 * b)
    LT1[H:, :H] = blk(1.5 * b)
    LT1[H:, H:] = blk(1.5 * ab + 2.5 * b)
    LT2 = np.zeros((2 * H, 2 * H))
    LT2[:H, :H] = blk(1.5 * ab)
    LT2[:H, H:] = blk(1.5 * a2b)
    LT2[H:, :H] = blk(-3.0 * ab)
    LT2[H:, H:] = blk(-3.0 * a2b)
    LB = np.zeros((2 * H, 2 * H))
    LB[:H, H:] = np.eye(H)
    LO = np.zeros((2 * H, 2))
    LO[:H, 0] = wout
    LO[H:, 1] = wout
    return {"in_LH": LH.astype(np.float32),
            "in_LB": LB.astype(np.float32),
            "in_LT1": LT1.astype(gdt),
            "in_LT2": LT2.astype(gdt),
            "in_LO": LO.astype(np.float32)}



def _host_precompute(x, W_in, b_in, W_hh, W_ih, bias, tau, W_out, b_out):
    x = np.asarray(x, dtype=np.float32)
    W_in = np.asarray(W_in, dtype=np.float32)
    b_in = np.asarray(b_in, dtype=np.float32)
    W_hh = np.asarray(W_hh, dtype=np.float32)
    W_ih = np.asarray(W_ih, dtype=np.float32)
    bias = np.asarray(bias, dtype=np.float32)
    tau = np.asarray(tau, dtype=np.float32)
    W_out = np.asarray(W_out, dtype=np.float32)

    W_comb = W_ih @ W_in                      # [H, BIN]
    b_comb = W_ih @ b_in + bias               # [H]
    C = x @ W_comb.T + b_comb                 # [B_FULL, S, H] f32

    t = np.linspace(0.0, 1.0, S).astype(np.float32)
    dt = np.float64(t[1]) - np.float64(t[0])
    d = 1.0 / tau.astype(np.float64)
    a = np.exp(-d * dt)
    b = 1.0 - a

    Wp = (1.5 * b[:, None] * W_hh.astype(np.float64)).T   # lhsT [k, j]
    Wm = (-0.5 * b[:, None] * W_hh.astype(np.float64)).T
    wout = W_out[0].astype(np.float64)                    # [H]

    Aev = np.zeros((2 * H, H + 1), np.float64)
    Aev[:H, :H] = Wp
    Aev[H:, :H] = Wm
    Aev[:H, H] = wout
    Aod = np.zeros((2 * H, H + 1), np.float64)
    Aod[:H, :H] = Wm
    Aod[H:, :H] = Wp
    Aod[H:, H] = wout
    # tail round index S (=1024, even): th_S lives in half S%2
    Atl = np.zeros((2 * H, H + 1), np.float64)
    if S % 2 == 0:
        Atl[:H, H] = wout
    else:
        Atl[H:, H] = wout
    Db = np.zeros((H, H + 1), np.float64)
    Db[:, :H] = np.diag(b)
    Da = np.diag(a)

    return C, {
        "in_Aev": Aev.astype(np.float32),
        "in_Aod": Aod.astype(np.float32),
        "in_Atl": Atl.astype(np.float32),
        "in_Db": Db.astype(np.float32),
        "in_Da": Da.astype(np.float32),
    }


def kernel(x, W_in, b_in, W_hh, W_ih, bias, tau, W_out, b_out):
    C, wmaps = _host_precompute(x, W_in, b_in, W_hh, W_ih, bias, tau,
                                W_out, b_out)
    b_out = np.asarray(b_out, dtype=np.float32)

    if SCHEME in ("pair", "pairz", "v7"):
        t = np.linspace(0.0, 1.0, S).astype(np.float32)
        dt = np.float64(t[1]) - np.float64(t[0])
        d = 1.0 / np.asarray(tau, dtype=np.float32).astype(np.float64)
        a = np.exp(-d * dt)
        b = 1.0 - a
        if SCHEME == "v7":
            wmaps = _v7_weights(a, b, np.asarray(W_hh, np.float32),
                                np.asarray(W_out, np.float32))
            builder = _build_program_v7
        elif SCHEME == "pairz":
            wmaps = _pairz_weights(a, b, np.asarray(W_hh, np.float32),
                                   np.asarray(W_out, np.float32))
            builder = _build_program_pairz
        else:
            wmaps = _pair_weights(a, b, np.asarray(W_hh, np.float32),
                                  np.asarray(W_out, np.float32))
            builder = _build_program_pair
            # prescaled pair C'': tile r = [b*c_{2r+1} ; (b/a)*c_{2r}]
            bf = b.astype(np.float32)[None, :]
            baf = (b / a).astype(np.float32)[None, :]
    else:
        builder = _build_program

    if "nc" not in _cached:
        _cached["nc"] = builder()
    nc = _cached["nc"]

    in_maps = []
    for i in range(N_CORES):
        Cc = C[i * B:(i + 1) * B]                        # [B, S, H]
        if SCHEME == "v7":
            Cb, C2 = _v7_cc(Cc, a, b)
            in_maps.append({"in_Cb": Cb, "in_C2": C2, **wmaps})
            continue
        if SCHEME == "pairz":
            C_core = _pairz_cc(Cc, a, b)                 # [NPAIR, 2H, 2H]
        elif SCHEME == "pair":
            odd = (Cc[:, 1::2, :] * bf).transpose(1, 2, 0)   # [NPAIR, H, B]
            even = (Cc[:, 0::2, :] * baf).transpose(1, 2, 0)
            C_core = np.ascontiguousarray(
                np.concatenate([odd, even], axis=1))     # [NPAIR, 2H, B]
        else:
            C_core = np.ascontiguousarray(Cc.transpose(1, 2, 0))  # [S, H, B]
        in_maps.append({"in_C": C_core, **wmaps})

    core_ids = list(range(N_CORES))
    _cached["in_maps"] = in_maps
    res = run_bass_kernel_spmd(nc, in_maps, core_ids)

    out = np.empty((B_FULL, S, 1), dtype=np.float32)
    for i in range(N_CORES):
        if SCHEME == "v7":
            dev = res.results[i]["out"].reshape(NGRP, 2, GRP, B)
            out[i * B:(i + 1) * B, :, 0] = (
                dev.transpose(3, 0, 2, 1).reshape(B, S) + b_out[0])
            continue
        if SCHEME in ("pair", "pairz"):
            dev = res.results[i]["out"].reshape(NSEGP, 2, SEGP, B)
            dev = dev.transpose(0, 2, 1, 3).reshape(S, B)   # [o, b]
        else:
            dev = res.results[i]["out"].reshape(S, B)        # [s, b_local]
        out[i * B:(i + 1) * B, :, 0] = dev.T + b_out[0]
    return out


def _in_maps_for_test(C, wmaps):
    maps = []
    for i in range(N_CORES):
        C_core = np.ascontiguousarray(C[i * B:(i + 1) * B].transpose(1, 2, 0))
        maps.append({"in_C": C_core, **wmaps})
    return maps



# revision 15
# speedup vs baseline: 1.0468x; 1.0468x over previous
"""Trainium2 Bass kernel for nn_LiquidNeuralNetwork (B=512, S=1024, IN=16, HID=64).

Strategy
--------
The reference integrates dh/dt = (-h + tanh(h) @ W_hh.T + inp + bias) / tau
with RK4 x 4 substeps per timestep (16 sequential tanh+matmul rounds per
step).  At dt = 1/1023 the integration error of far cheaper schemes is orders
of magnitude below f32 rounding noise, so we integrate the same ODE with an
exponential integrator + AB2 extrapolation of the (tiny) tanh coupling term:

    H_s = a*H_{s-1} + b*(c_s + 1.5*g_{s-1} - 0.5*g_{s-2}),
    g_s = W_hh @ tanh(H_s),  a = exp(-dt/tau), b = 1 - a,
    c_s = W_ih @ (W_in x_s + b_in) + bias   (precomputed, hidden-major)

which agrees with the reference to ~6e-6 (the f32 noise floor of the
reference itself) while needing ONE tanh + matmul round per timestep.

On-device layout: hidden on partitions, batch on free dim; batch sharded
8 ways (64 per core).  Per round the PSUM bank accumulates the full affine
update via matmuls only:

    bank_r[0:64]  = diag(b) @ c_r + diag(a) @ hm_r + [Wp;Wm] @ [th_r;th_{r-1}]
    bank_r[64]    = W_out @ th_r          (the per-step scalar output)

with Wp = (1.5*b*W_hh)^T, Wm = (-0.5*b*W_hh)^T.  tanh runs on ACT straight
from PSUM; DVE copies bank->SBUF (h materialization + output-row collection);
everything except ACT->PE->ACT is off the critical path.
"""

import os
import numpy as np

import concourse.bacc as bacc
import concourse.tile as tile
from concourse import mybir
from concourse.bass_utils import run_bass_kernel_spmd

F32 = mybir.dt.float32
H = 64          # hidden
BIN = 16        # input features
B_FULL = 512
S = int(os.environ.get("LNN_S", "1024"))   # harness always runs 1024
N_CORES = 8
B = B_FULL // N_CORES   # 64 per-core batch
SEG = 128 if S % 128 == 0 else S           # output segment length (steps)
N_SEG = S // SEG

TRACE = bool(int(os.environ.get("LNN_TRACE", "0")))
SCHEME = os.environ.get("LNN_SCHEME", "v7")   # "v7" | "pair" | "pairz" | "e2"

GRP = 8                       # pair-slots per bulk output matmul (v7)
NGRP = (S // 2) // GRP

NPAIR = S // 2                 # pair rounds
SEGP = NPAIR if NPAIR <= 256 else 256   # pair-slots per output segment
NSEGP = NPAIR // SEGP

_cached = {}


def _build_program():
    """Build + compile the Bass program (same NEFF for all cores)."""
    nc = bacc.Bacc("TRN2", target_bir_lowering=False, debug=False)

    in_C = nc.dram_tensor("in_C", (S, H, B), F32, kind="ExternalInput").ap()
    in_Aev = nc.dram_tensor("in_Aev", (2 * H, H + 1), F32, kind="ExternalInput").ap()
    in_Aod = nc.dram_tensor("in_Aod", (2 * H, H + 1), F32, kind="ExternalInput").ap()
    in_Atl = nc.dram_tensor("in_Atl", (2 * H, H + 1), F32, kind="ExternalInput").ap()
    in_Db = nc.dram_tensor("in_Db", (H, H + 1), F32, kind="ExternalInput").ap()
    in_Da = nc.dram_tensor("in_Da", (H, H), F32, kind="ExternalInput").ap()
    out_dram = nc.dram_tensor("out", (N_SEG, SEG * B), F32, kind="ExternalOutput").ap()

    TANH = mybir.ActivationFunctionType.Tanh

    with tile.TileContext(nc) as tc:
        with (
            tc.tile_pool(name="wts", bufs=1) as wts,
            tc.tile_pool(name="thp", bufs=1) as thp,
            tc.tile_pool(name="osb", bufs=2) as osbp,
            tc.tile_pool(name="cp", bufs=10) as cp,
            tc.tile_pool(name="hmp", bufs=3) as hmp,
            tc.tile_pool(name="hbank", bufs=4, space="PSUM") as hbank,
        ):
            t_Aev = wts.tile([2 * H, H + 1], F32, tag="aev")
            t_Aod = wts.tile([2 * H, H + 1], F32, tag="aod")
            t_Atl = wts.tile([2 * H, H + 1], F32, tag="atl")
            t_Db = wts.tile([H, H + 1], F32, tag="db")
            t_Da = wts.tile([H, H], F32, tag="da")
            nc.sync.dma_start(out=t_Aev, in_=in_Aev)
            nc.sync.dma_start(out=t_Aod, in_=in_Aod)
            nc.sync.dma_start(out=t_Atl, in_=in_Atl)
            nc.sync.dma_start(out=t_Db, in_=in_Db)
            nc.sync.dma_start(out=t_Da, in_=in_Da)

            # persistent tanh tile: half0 = th of even rounds, half1 = odd
            t_th = thp.tile([2 * H, B], F32, tag="th")
            nc.vector.memset(t_th, 0.0)

            # output staging: only partition 64 is used; slot o at free
            # offset (o % SEG)*B.  Two tiles ping-pong across segments.
            t_osb = [osbp.tile([H + 1, SEG * B], F32, tag="osb", name=f"t_osb{i}")
                     for i in range(2)]

            prev_bank = None
            for r in range(1, S):
                t_c = cp.tile([H, B], F32, tag="c")
                nc.sync.dma_start(out=t_c, in_=in_C[r])

                bank = hbank.tile([H + 1, B], F32, tag="bank")
                last = r == 1
                # M4 first (start=True): clears rows 0..64 (col H of Db is 0)
                nc.tensor.matmul(bank, t_Db, t_c, start=True, stop=last)

                if r >= 2:
                    o = r - 2          # output index evacuated this round
                    seg, slot = divmod(o, SEG)
                    # evacuate prev bank's output row (lane-aligned copy)
                    nc.vector.tensor_copy(
                        t_osb[seg % 2][H:H + 1, slot * B:(slot + 1) * B],
                        prev_bank[H:H + 1, :],
                    )
                    if slot == SEG - 1:
                        nc.sync.dma_start(
                            out=out_dram[seg],
                            in_=t_osb[seg % 2][H:H + 1, :],
                        )
                    # h materialization for the decay term
                    t_hm = hmp.tile([H, B], F32, tag="hm")
                    nc.vector.tensor_copy(t_hm, prev_bank[:H, :])
                    # tanh straight from PSUM into this round's th half
                    half = r % 2
                    nc.scalar.activation(
                        t_th[half * H:(half + 1) * H, :], prev_bank[:H, :], TANH)
                    nc.tensor.matmul(bank[:H, :], t_Da, t_hm,
                                     start=False, stop=False)
                    t_A = t_Aev if r % 2 == 0 else t_Aod
                    nc.tensor.matmul(bank, t_A, t_th, start=False, stop=True)
                prev_bank = bank

            # tail: evacuate out_{S-2}; th_S = tanh(H_{S-1}); out_{S-1}
            o = S - 2
            seg, slot = divmod(o, SEG)
            nc.vector.tensor_copy(
                t_osb[seg % 2][H:H + 1, slot * B:(slot + 1) * B],
                prev_bank[H:H + 1, :],
            )
            half = S % 2
            nc.scalar.activation(
                t_th[half * H:(half + 1) * H, :], prev_bank[:H, :], TANH)
            tbank = hbank.tile([H + 1, B], F32, tag="bank")
            nc.tensor.matmul(tbank, t_Atl, t_th, start=True, stop=True)
            o = S - 1
            seg, slot = divmod(o, SEG)
            nc.vector.tensor_copy(
                t_osb[seg % 2][H:H + 1, slot * B:(slot + 1) * B],
                tbank[H:H + 1, :],
            )
            nc.sync.dma_start(out=out_dram[seg], in_=t_osb[seg % 2][H:H + 1, :])

    nc.compile()
    return nc


def _build_program_v7():
    """Pair scheme v7: no f32 matmuls, one bf16 tanh/round, bulk output.

    State per round r (2 timesteps): P = [H_s; H_{s+1}] and the prescaled
    decay copy Q = [a*H_{s+1}; a^2*H_{s+1}], both f32 PSUM [2H, B].

        P_r = Q_{r-1} + Cb_r + LT1 @ T1_r        (DVE base + 1 bf16 matmul)
        Q_r = a^2*Q_{r-1} + C2_r + LT12 @ T1_r   (DVE mul+add + 1 bf16 matmul)
        T1_r = tanh(P_{r-1})  (single bf16 ACT, written into a group buffer)

    Cb/C2 are host-premixed bf16 tiles DMA'd on the Sync and ACT hwdge
    queues respectively.  Outputs: T1 tiles accumulate in a [2H, GRP*B]
    group buffer; every GRP rounds one bulk matmul LO @ thbuf produces
    [2, GRP*B] in PSUM, evacuated by GpSimd and DMA'd out via swdge.
    """
    nc = bacc.Bacc("TRN2", target_bir_lowering=False, debug=False)

    BF16 = mybir.dt.bfloat16
    NP = NPAIR

    NCHUNK = NP // GRP
    in_Cb = nc.dram_tensor("in_Cb", (NCHUNK, 2 * H, GRP * B), BF16,
                           kind="ExternalInput").ap()
    in_C2 = nc.dram_tensor("in_C2", (NCHUNK, 2 * H, GRP * B), BF16,
                           kind="ExternalInput").ap()
    in_LT1 = nc.dram_tensor("in_LT1", (2 * H, 2 * H), BF16,
                            kind="ExternalInput").ap()
    in_LT12 = nc.dram_tensor("in_LT12", (2 * H, 2 * H), BF16,
                             kind="ExternalInput").ap()
    in_LO = nc.dram_tensor("in_LO", (2 * H, 2), BF16,
                           kind="ExternalInput").ap()
    in_a2 = nc.dram_tensor("in_a2", (2 * H, 1), F32, kind="ExternalInput").ap()
    out_dram = nc.dram_tensor("out", (NGRP, 2, GRP * B), F32,
                              kind="ExternalOutput").ap()

    TANH = mybir.ActivationFunctionType.Tanh

    with tile.TileContext(nc) as tc:
        with (
            tc.tile_pool(name="wts", bufs=1) as wts,
            tc.tile_pool(name="cbp", bufs=4) as cbp,
            tc.tile_pool(name="c2p", bufs=4) as c2p,
            tc.tile_pool(name="tmpp", bufs=3) as tmpp,
            tc.tile_pool(name="thb", bufs=2) as thbp,
            tc.tile_pool(name="osb", bufs=2) as osbp,
            tc.tile_pool(name="pP", bufs=3, space="PSUM") as pP,
            tc.tile_pool(name="pQ", bufs=3, space="PSUM") as pQ,
            tc.tile_pool(name="pO", bufs=2, space="PSUM") as pO,
        ):
            t_LT1 = wts.tile([2 * H, 2 * H], BF16, name="t_LT1")
            t_LT12 = wts.tile([2 * H, 2 * H], BF16, name="t_LT12")
            t_LO = wts.tile([2 * H, 2], BF16, name="t_LO")
            t_a2 = wts.tile([2 * H, 1], F32, name="t_a2")
            nc.sync.dma_start(out=t_LT1, in_=in_LT1)
            nc.sync.dma_start(out=t_LT12, in_=in_LT12)
            nc.sync.dma_start(out=t_LO, in_=in_LO)
            nc.sync.dma_start(out=t_a2, in_=in_a2)

            t_thb = [thbp.tile([2 * H, GRP * B], BF16, tag="thb",
                               name=f"t_thb{i}") for i in range(2)]

            # chunked c-tile DMA: one [2H, GRP*B] transfer per GRP rounds
            # per stream, both on the Sync hwdge queue; prefetch 2 chunks.
            cb_t, c2_t = {}, {}

            def fetch(k):
                if k >= NCHUNK:
                    return
                cb_t[k] = cbp.tile([2 * H, GRP * B], BF16, tag="cb",
                                   name=f"cb{k % 4}")
                nc.sync.dma_start(out=cb_t[k], in_=in_Cb[k])
                c2_t[k] = c2p.tile([2 * H, GRP * B], BF16, tag="c2",
                                   name=f"c2{k % 4}")
                nc.sync.dma_start(out=c2_t[k], in_=in_C2[k])

            for k in range(3):
                fetch(k)

            # boot: P0 = Cb[0], Q0 = C2[0]  (H_0 = 0)
            P = pP.tile([2 * H, B], F32, tag="P")
            Q = pQ.tile([2 * H, B], F32, tag="Q")
            nc.vector.tensor_copy(P, cb_t[0][:, 0:B])
            nc.vector.tensor_copy(Q, c2_t[0][:, 0:B])

            prev_P, prev_Q = P, Q
            for r in range(1, NP):
                g, slot = divmod(r - 1, GRP)
                ck, cs = divmod(r, GRP)
                t_cb = cb_t[ck][:, cs * B:(cs + 1) * B]
                t_c2 = c2_t[ck][:, cs * B:(cs + 1) * B]

                # single bf16 tanh straight from PSUM into the group buffer
                T1 = t_thb[g % 2][:, slot * B:(slot + 1) * B]
                nc.scalar.activation(T1, prev_P, TANH)
                if cs == 1:
                    fetch(ck + 2)

                P = pP.tile([2 * H, B], F32, tag="P")
                Q = pQ.tile([2 * H, B], F32, tag="Q")
                # P base on DVE; Q decay scale on ACT, Q base add on DVE
                nc.vector.tensor_add(P, prev_Q, t_cb)
                t_tmp = tmpp.tile([2 * H, B], F32, tag="tmp")
                nc.vector.tensor_scalar_mul(t_tmp, prev_Q, t_a2)
                nc.vector.tensor_add(Q, t_tmp, t_c2)

                # coupling matmuls accumulate onto the DVE-written bases
                nc.tensor.matmul(P, t_LT1, T1, start=False, stop=True,
                                 skip_group_check=True)
                nc.tensor.matmul(Q, t_LT12, T1, start=False, stop=True,
                                 skip_group_check=True)

                if slot == GRP - 1:
                    ob = pO.tile([2, GRP * B], F32, tag="ob")
                    nc.tensor.matmul(ob, t_LO, t_thb[g % 2],
                                     start=True, stop=True)
                    t_os = osbp.tile([2, GRP * B], F32, tag="os")
                    nc.vector.tensor_copy(t_os, ob)
                    nc.gpsimd.dma_start(out=out_dram[g], in_=t_os)

                prev_P, prev_Q = P, Q

            # tail: T1(NP) completes the last group
            g, slot = NGRP - 1, GRP - 1
            T1 = t_thb[g % 2][:, slot * B:(slot + 1) * B]
            nc.scalar.activation(T1, prev_P, TANH)
            ob = pO.tile([2, GRP * B], F32, tag="ob")
            nc.tensor.matmul(ob, t_LO, t_thb[g % 2], start=True, stop=True)
            t_os = osbp.tile([2, GRP * B], F32, tag="os")
            nc.vector.tensor_copy(t_os, ob)
            nc.gpsimd.dma_start(out=out_dram[g], in_=t_os)

    nc.compile()
    return nc


def _v7_weights(a, b, W_hh, W_out):
    """LT1/LT12 as in pairz, plus bf16 LO and the a^2 decay vector."""
    import ml_dtypes
    wm = _pairz_weights(a, b, W_hh, W_out)
    return {
        "in_LT1": wm["in_LT1"],
        "in_LT12": wm["in_LT12"],
        "in_LO": wm["in_LO"].astype(ml_dtypes.bfloat16),
        "in_a2": wm["in_a2"],
    }


def _v7_cc(Cc, a, b):
    """Host Cb/C2 tiles [NPAIR, 2H, B] bf16 for one core (see _pairz_cc)."""
    import ml_dtypes
    ab = a * b
    ce = Cc[:, 0::2, :].astype(np.float64)    # c_{2r}   [B, NPAIR, H]
    co = Cc[:, 1::2, :].astype(np.float64)    # c_{2r+1}
    z = ab * ce + b * co
    Bc = Cc.shape[0]
    Cb = np.empty((NPAIR, 2 * H, Bc), np.float32)
    C2 = np.empty((NPAIR, 2 * H, Bc), np.float32)
    Cb[:, :H] = (b * ce).transpose(1, 2, 0)
    Cb[:, H:] = z.transpose(1, 2, 0)
    C2[:, :H] = (a * z).transpose(1, 2, 0)
    C2[:, H:] = (a * a * z).transpose(1, 2, 0)
    c1 = co[:, 0, :]                          # [B, H]
    Cb[0, :H] = 0.0
    Cb[0, H:] = (b * c1).T
    C2[0, :H] = (ab * c1).T
    C2[0, H:] = (a * ab * c1).T

    def chunk(arr):
        return np.ascontiguousarray(
            arr.reshape(NPAIR // GRP, GRP, 2 * H, Bc)
            .transpose(0, 2, 1, 3)
            .reshape(NPAIR // GRP, 2 * H, GRP * Bc)
        ).astype(ml_dtypes.bfloat16)

    return chunk(Cb), chunk(C2)


def _build_program_pairz():
    """Pair scheme v6 ("zlite"): 2 steps per round, NO f32 matmuls on PE.

    One PSUM tile P_r [128,128] per round: cols 0:64 "bank" = [H_s; H_{s+1}],
    cols 64:128 "bank2" = [a*H_{s+1}; a^2*H_{s+1}] (pre-scaled decay copies,
    maintained so the next round's injections are lane-aligned DVE ops):

        bank_r   = bank2_{r-1} + Cb''_r + coupling(th)          (1 DVE add)
        bank2_r  = a^2*bank2_{r-1} + C2''_r + coupling2(th)     (mul + add)

    with all c-terms host-premixed into C''.  PE does only: 2 bf16 coupling
    matmuls (accumulating onto the DVE-written base via start=False) and the
    f32 output matvec.  tanh pair: bf16 (coupling) + f32 (out matvec).
    """
    nc = bacc.Bacc("TRN2", target_bir_lowering=False, debug=False)

    BF16 = mybir.dt.bfloat16

    in_C = nc.dram_tensor("in_C", (NPAIR, 2 * H, 2 * H), F32,
                          kind="ExternalInput").ap()
    ins = {}
    for nm in ("LT1", "LT12"):
        ins[nm] = nc.dram_tensor(f"in_{nm}", (2 * H, 2 * H), BF16,
                                 kind="ExternalInput").ap()
    ins["LO"] = nc.dram_tensor("in_LO", (2 * H, 2), F32,
                               kind="ExternalInput").ap()
    in_a2 = nc.dram_tensor("in_a2", (2 * H, 1), F32, kind="ExternalInput").ap()
    out_dram = nc.dram_tensor("out", (NSEGP, 2, SEGP * B), F32,
                              kind="ExternalOutput").ap()

    TANH = mybir.ActivationFunctionType.Tanh

    with tile.TileContext(nc) as tc:
        with (
            tc.tile_pool(name="wts", bufs=1) as wts,
            tc.tile_pool(name="thp", bufs=3) as thp,
            tc.tile_pool(name="thf", bufs=3) as thfp,
            tc.tile_pool(name="osb", bufs=2) as osbp,
            tc.tile_pool(name="ccp", bufs=6) as ccp,
            tc.tile_pool(name="tmpp", bufs=3) as tmpp,
            tc.tile_pool(name="pbank", bufs=4, space="PSUM") as pbank,
            tc.tile_pool(name="obank", bufs=3, space="PSUM") as obankp,
        ):
            t_w = {}
            for nm in ("LT1", "LT12"):
                t_w[nm] = wts.tile([2 * H, 2 * H], BF16, name=f"t_{nm}")
                nc.sync.dma_start(out=t_w[nm], in_=ins[nm])
            t_w["LO"] = wts.tile([2 * H, 2], F32, name="t_LO")
            nc.sync.dma_start(out=t_w["LO"], in_=ins["LO"])
            t_a2 = wts.tile([2 * H, 1], F32, name="t_a2")
            nc.sync.dma_start(out=t_a2, in_=in_a2)

            t_osb = [osbp.tile([2, SEGP * B], F32, tag="osb", name=f"t_osb{i}")
                     for i in range(2)]

            # boot: P_0 = C''_0 (H_0 = 0 so no decay/coupling terms)
            t_cc = ccp.tile([2 * H, 2 * H], F32, tag="cc")
            nc.sync.dma_start(out=t_cc, in_=in_C[0])
            P = pbank.tile([2 * H, 2 * H], F32, tag="P")
            nc.vector.tensor_copy(P, t_cc)

            prev_P = P
            prev_thf = None
            pending = []

            def flush_one():
                ob_t, m = pending.pop(0)
                seg, slot = divmod(m, SEGP)
                nc.vector.tensor_copy(
                    t_osb[seg % 2][0:2, slot * B:(slot + 1) * B], ob_t)
                if slot == SEGP - 1:
                    nc.sync.dma_start(out=out_dram[seg],
                                      in_=t_osb[seg % 2][0:2, :])

            for r in range(1, NPAIR):
                t_cc = ccp.tile([2 * H, 2 * H], F32, tag="cc")
                nc.sync.dma_start(out=t_cc, in_=in_C[r])

                P = pbank.tile([2 * H, 2 * H], F32, tag="P")
                # critical-path injection: bank base = bank2_prev + Cb''
                nc.vector.tensor_add(P[:, :2 * H - H], prev_P[:, H:H + H],
                                     t_cc[:, 0:H])
                # off-path: bank2 base = a^2*bank2_prev + C2''
                t_tmp = tmpp.tile([2 * H, H], F32, tag="tmp")
                nc.vector.tensor_scalar_mul(t_tmp, prev_P[:, H:H + H], t_a2)
                nc.vector.tensor_add(P[:, H:H + H], t_tmp, t_cc[:, H:H + H])

                # tanh pair from prev bank
                T1 = thp.tile([2 * H, B], BF16, tag="t1")
                nc.scalar.activation(T1, prev_P[:, 0:H], TANH)
                t_thf = thfp.tile([2 * H, B], F32, tag="thf")
                nc.scalar.activation(t_thf, prev_P[:, 0:H], TANH)

                # previous round's out matvec (f32) while ACT runs
                if prev_thf is not None:
                    ob = obankp.tile([2, B], F32, tag="ob")
                    nc.tensor.matmul(ob, t_w["LO"], prev_thf,
                                     start=True, stop=True)
                    pending.append((ob, r - 2))
                if len(pending) > 1:
                    flush_one()

                # coupling matmuls accumulate onto the DVE-written base
                nc.tensor.matmul(P[:, 0:H], t_w["LT1"], T1,
                                 start=False, stop=False,
                                 skip_group_check=True)
                nc.tensor.matmul(P[:, H:H + H], t_w["LT12"], T1,
                                 start=False, stop=True,
                                 skip_group_check=True)

                prev_P, prev_thf = P, t_thf

            # tail
            ob = obankp.tile([2, B], F32, tag="ob")
            nc.tensor.matmul(ob, t_w["LO"], prev_thf, start=True, stop=True)
            pending.append((ob, NPAIR - 2))
            t_thf = thfp.tile([2 * H, B], F32, tag="thf")
            nc.scalar.activation(t_thf, prev_P[:, 0:H], TANH)
            ob = obankp.tile([2, B], F32, tag="ob")
            nc.tensor.matmul(ob, t_w["LO"], t_thf, start=True, stop=True)
            pending.append((ob, NPAIR - 1))
            while pending:
                flush_one()

    nc.compile()
    return nc


def _pairz_weights(a, b, W_hh, W_out):
    """Host lhsT matrices + per-partition a^2 vector for the v6 scheme."""
    import ml_dtypes
    W = W_hh.astype(np.float64)
    wout = W_out[0].astype(np.float64)
    ab = a * b

    def blk(v):
        return (v[:, None] * W).T

    cp1, cm1 = 1.5 * b, -0.5 * b
    cp2, cm2 = 1.5 * ab + 2.5 * b, -0.5 * ab - 1.5 * b

    LT1 = np.zeros((2 * H, 2 * H))
    LT1[:H, :H] = blk(cm1)
    LT1[H:, :H] = blk(cp1)
    LT1[:H, H:] = blk(cm2)
    LT1[H:, H:] = blk(cp2)
    LT12 = np.zeros((2 * H, 2 * H))
    LT12[:H, :H] = blk(a * cm2)
    LT12[H:, :H] = blk(a * cp2)
    LT12[:H, H:] = blk(a * a * cm2)
    LT12[H:, H:] = blk(a * a * cp2)
    LO = np.zeros((2 * H, 2))
    LO[:H, 0] = wout
    LO[H:, 1] = wout
    a2v = np.concatenate([a * a, a * a]).reshape(2 * H, 1)
    return {"in_LT1": LT1.astype(ml_dtypes.bfloat16),
            "in_LT12": LT12.astype(ml_dtypes.bfloat16),
            "in_LO": LO.astype(np.float32),
            "in_a2": a2v.astype(np.float32)}


def _pairz_cc(Cc, a, b):
    """Host C'' quadrant tiles [NPAIR, 2H, 2H] for one core.

    Cc: [B, S, H] raw input-current.  Quadrants (rows x cols):
      [:, :H]  (bank col):  [b*c_s ; ab*c_s + b*c_{s+1}]
      [:, H:]  (bank2 col): [a^2 b*c_s + ab*c_{s+1} ; a^3 b*c_s + a^2 b*c_{s+1}]
    Boot tile (r=0, H_0=0): bank col = [0 ; b*c_1],
      bank2 col = [ab*c_1 ; a^2 b*c_1].
    """
    ab = a * b
    ce = Cc[:, 0::2, :].astype(np.float64)    # c_{2r}   [B, NPAIR, H]
    co = Cc[:, 1::2, :].astype(np.float64)    # c_{2r+1}
    out = np.empty((NPAIR, 2 * H, 2 * H), np.float32)
    # bank col
    out[:, :H, :H] = (b * ce).transpose(1, 2, 0)
    out[:, H:, :H] = (ab * ce + b * co).transpose(1, 2, 0)
    # bank2 col
    out[:, :H, H:] = (a * (ab * ce + b * co)).transpose(1, 2, 0)
    out[:, H:, H:] = (a * a * (ab * ce + b * co)).transpose(1, 2, 0)
    # boot overrides (c_0 unused, H_0 = 0)
    c1 = co[:, 0, :]                          # [B, H]
    out[0, :H, :H] = 0.0
    out[0, H:, :H] = (b * c1).T
    out[0, :H, H:] = (ab * c1).T
    out[0, H:, H:] = (a * ab * c1).T
    return out


def _build_program_pair():
    """Pair-corrected scheme v2: 2 timesteps per tanh round (S/2 rounds).

    PSUM bank halves = [H_s ; H_{s+1}^pred]; one bf16 ACT tanh covers both
    and feeds the (tiny) tanh-coupling matmuls LT1/LT2 in bf16; a second f32
    tanh feeds the f32 output matvec.  The c-injection is folded into the
    f32 decay matmul LH via a host-prescaled C'' tile DMA'd into the hm
    tile, whose lower half gets H_{s-1} added by one DVE op:
        hm = [b*c_{s+1} ; (b/a)*c_s + H_{s-1}]
        LH @ hm = [a*H_{s-1}+b*c_s ; a^2*H_{s-1}+ab*c_s+b*c_{s+1}]
    """
    nc = bacc.Bacc("TRN2", target_bir_lowering=False, debug=False)

    BF16 = mybir.dt.bfloat16
    GDT = BF16 if os.environ.get("LNN_GDT", "bf16") == "bf16" else F32

    in_C = nc.dram_tensor("in_C", (NPAIR, 2 * H, B), F32,
                          kind="ExternalInput").ap()
    ins = {}
    for nm in ("LH", "LB"):
        ins[nm] = nc.dram_tensor(f"in_{nm}", (2 * H, 2 * H), F32,
                                 kind="ExternalInput").ap()
    for nm in ("LT1", "LT2"):
        ins[nm] = nc.dram_tensor(f"in_{nm}", (2 * H, 2 * H), GDT,
                                 kind="ExternalInput").ap()
    ins["LO"] = nc.dram_tensor("in_LO", (2 * H, 2), F32,
                               kind="ExternalInput").ap()
    out_dram = nc.dram_tensor("out", (NSEGP, 2, SEGP * B), F32,
                              kind="ExternalOutput").ap()

    TANH = mybir.ActivationFunctionType.Tanh

    with tile.TileContext(nc) as tc:
        with (
            tc.tile_pool(name="wts", bufs=1) as wts,
            tc.tile_pool(name="thp", bufs=4) as thp,
            tc.tile_pool(name="thf", bufs=3) as thfp,
            tc.tile_pool(name="thz", bufs=1) as thz,
            tc.tile_pool(name="osb", bufs=2) as osbp,
            tc.tile_pool(name="hmp", bufs=8) as hmp,
            tc.tile_pool(name="hbank", bufs=4, space="PSUM") as hbank,
            tc.tile_pool(name="obank", bufs=3, space="PSUM") as obankp,
        ):
            t_w = {}
            for nm in ("LH", "LB"):
                t_w[nm] = wts.tile([2 * H, 2 * H], F32, name=f"t_{nm}")
                nc.sync.dma_start(out=t_w[nm], in_=ins[nm])
            for nm in ("LT1", "LT2"):
                t_w[nm] = wts.tile([2 * H, 2 * H], GDT, name=f"t_{nm}")
                nc.sync.dma_start(out=t_w[nm], in_=ins[nm])
            t_w["LO"] = wts.tile([2 * H, 2], F32, name="t_LO")
            nc.sync.dma_start(out=t_w["LO"], in_=ins["LO"])

            t_zero = thz.tile([2 * H, B], GDT, tag="t1zero")
            nc.vector.memset(t_zero, 0.0)
            t_osb = [osbp.tile([2, SEGP * B], F32, tag="osb", name=f"t_osb{i}")
                     for i in range(2)]

            # boot: bank_0 = [0 ; b*c_1]  (C''_0 half0 = b*c_1)
            t_hm = hmp.tile([2 * H, B], F32, tag="hm")
            nc.sync.dma_start(out=t_hm, in_=in_C[0])
            bank = hbank.tile([2 * H, B], F32, tag="bank")
            nc.tensor.matmul(bank, t_w["LB"], t_hm, start=True, stop=True)

            prev_bank = bank
            prev_T1 = t_zero
            prev_thf = None           # f32 tanh pair awaiting its out matvec
            pending = []              # [(ob_tile, slot_index)] not yet evac'd

            def flush_one():
                ob_t, m = pending.pop(0)
                seg, slot = divmod(m, SEGP)
                nc.vector.tensor_copy(
                    t_osb[seg % 2][0:2, slot * B:(slot + 1) * B], ob_t)
                if slot == SEGP - 1:
                    nc.sync.dma_start(out=out_dram[seg],
                                      in_=t_osb[seg % 2][0:2, :])

            for r in range(1, NPAIR):
                t_hm = hmp.tile([2 * H, B], F32, tag="hm")
                nc.sync.dma_start(out=t_hm, in_=in_C[r])

                bank = hbank.tile([2 * H, B], F32, tag="bank")
                # bf16 matmul first (FWL-friendly after last round's bf16 LT1)
                nc.tensor.matmul(bank, t_w["LT2"], prev_T1,
                                 start=True, stop=False)
                # the two f32 matmuls adjacent: previous round's out matvec,
                # then the decay+input injection
                if prev_thf is not None:
                    ob = obankp.tile([2, B], F32, tag="ob")
                    nc.tensor.matmul(ob, t_w["LO"], prev_thf,
                                     start=True, stop=True)
                    pending.append((ob, r - 2))

                # tanh pair: bf16 for the coupling path (critical), f32 for
                # the output matvec (off critical path)
                T1 = thp.tile([2 * H, B], GDT, tag="t1")
                nc.scalar.activation(T1, prev_bank, TANH)
                t_thf = thfp.tile([2 * H, B], F32, tag="thf")
                nc.scalar.activation(t_thf, prev_bank, TANH)

                # hm lower half += H_{s-1} (from prev bank)
                nc.vector.tensor_add(t_hm[H:, :], t_hm[H:, :],
                                     prev_bank[H:, :])

                if len(pending) > 1:
                    flush_one()

                nc.tensor.matmul(bank, t_w["LH"], t_hm, start=False,
                                 stop=False)
                nc.tensor.matmul(bank, t_w["LT1"], T1, start=False, stop=True)

                prev_bank, prev_T1, prev_thf = bank, T1, t_thf

            # tail: emit out matvecs for the last two tanh pairs, flush all
            ob = obankp.tile([2, B], F32, tag="ob")
            nc.tensor.matmul(ob, t_w["LO"], prev_thf, start=True, stop=True)
            pending.append((ob, NPAIR - 2))
            t_thf = thfp.tile([2 * H, B], F32, tag="thf")
            nc.scalar.activation(t_thf, prev_bank, TANH)
            ob = obankp.tile([2, B], F32, tag="ob")
            nc.tensor.matmul(ob, t_w["LO"], t_thf, start=True, stop=True)
            pending.append((ob, NPAIR - 1))
            while pending:
                flush_one()   # final segment's DMA fires on its last slot

    nc.compile()
    return nc



def _pair_weights(a, b, W_hh, W_out):
    """Host lhsT matrices for the pair-corrected scheme (f64 in)."""
    import ml_dtypes
    gdt = (ml_dtypes.bfloat16 if os.environ.get("LNN_GDT", "bf16") == "bf16"
           else np.float32)
    W = W_hh.astype(np.float64)
    wout = W_out[0].astype(np.float64)
    ab, a2, a2b = a * b, a * a, a * a * b

    def blk(v):
        return (v[:, None] * W).T

    LH = np.zeros((2 * H, 2 * H))
    LH[:H, H:] = np.eye(H)
    LH[H:, :H] = np.diag(a)
    LH[H:, H:] = np.diag(a2)
    LT1 = np.zeros((2 * H, 2 * H))
    LT1[:H, :H] = blk(-0.5 * b + 1.5 * ab)
    LT1[:H, H:] = blk(-0.5 * ab + 1.5 * a2b - 1.5 * b)
    LT1[H:, :H] = blk(1.5 * b)
    LT1[H:, H:] = blk(1.5 * ab + 2.5 * b)
    LT2 = np.zeros((2 * H, 2 * H))
    LT2[:H, :H] = blk(1.5 * ab)
    LT2[:H, H:] = blk(1.5 * a2b)
    LT2[H:, :H] = blk(-3.0 * ab)
    LT2[H:, H:] = blk(-3.0 * a2b)
    LB = np.zeros((2 * H, 2 * H))
    LB[:H, H:] = np.eye(H)
    LO = np.zeros((2 * H, 2))
    LO[:H, 0] = wout
    LO[H:, 1] = wout
    return {"in_LH": LH.astype(np.float32),
            "in_LB": LB.astype(np.float32),
            "in_LT1": LT1.astype(gdt),
            "in_LT2": LT2.astype(gdt),
            "in_LO": LO.astype(np.float32)}



def _host_precompute(x, W_in, b_in, W_hh, W_ih, bias, tau, W_out, b_out):
    x = np.asarray(x, dtype=np.float32)
    W_in = np.asarray(W_in, dtype=np.float32)
    b_in = np.asarray(b_in, dtype=np.float32)
    W_hh = np.asarray(W_hh, dtype=np.float32)
    W_ih = np.asarray(W_ih, dtype=np.float32)
    bias = np.asarray(bias, dtype=np.float32)
    tau = np.asarray(tau, dtype=np.float32)
    W_out = np.asarray(W_out, dtype=np.float32)

    W_comb = W_ih @ W_in                      # [H, BIN]
    b_comb = W_ih @ b_in + bias               # [H]
    C = x @ W_comb.T + b_comb                 # [B_FULL, S, H] f32

    t = np.linspace(0.0, 1.0, S).astype(np.float32)
    dt = np.float64(t[1]) - np.float64(t[0])
    d = 1.0 / tau.astype(np.float64)
    a = np.exp(-d * dt)
    b = 1.0 - a

    Wp = (1.5 * b[:, None] * W_hh.astype(np.float64)).T   # lhsT [k, j]
    Wm = (-0.5 * b[:, None] * W_hh.astype(np.float64)).T
    wout = W_out[0].astype(np.float64)                    # [H]

    Aev = np.zeros((2 * H, H + 1), np.float64)
    Aev[:H, :H] = Wp
    Aev[H:, :H] = Wm
    Aev[:H, H] = wout
    Aod = np.zeros((2 * H, H + 1), np.float64)
    Aod[:H, :H] = Wm
    Aod[H:, :H] = Wp
    Aod[H:, H] = wout
    # tail round index S (=1024, even): th_S lives in half S%2
    Atl = np.zeros((2 * H, H + 1), np.float64)
    if S % 2 == 0:
        Atl[:H, H] = wout
    else:
        Atl[H:, H] = wout
    Db = np.zeros((H, H + 1), np.float64)
    Db[:, :H] = np.diag(b)
    Da = np.diag(a)

    return C, {
        "in_Aev": Aev.astype(np.float32),
        "in_Aod": Aod.astype(np.float32),
        "in_Atl": Atl.astype(np.float32),
        "in_Db": Db.astype(np.float32),
        "in_Da": Da.astype(np.float32),
    }


def kernel(x, W_in, b_in, W_hh, W_ih, bias, tau, W_out, b_out):
    C, wmaps = _host_precompute(x, W_in, b_in, W_hh, W_ih, bias, tau,
                                W_out, b_out)
    b_out = np.asarray(b_out, dtype=np.float32)

    if SCHEME in ("pair", "pairz", "v7"):
        t = np.linspace(0.0, 1.0, S).astype(np.float32)
        dt = np.float64(t[1]) - np.float64(t[0])
        d = 1.0 / np.asarray(tau, dtype=np.float32).astype(np.float64)
        a = np.exp(-d * dt)
        b = 1.0 - a
        if SCHEME == "v7":
            wmaps = _v7_weights(a, b, np.asarray(W_hh, np.float32),
                                np.asarray(W_out, np.float32))
            builder = _build_program_v7
        elif SCHEME == "pairz":
            wmaps = _pairz_weights(a, b, np.asarray(W_hh, np.float32),
                                   np.asarray(W_out, np.float32))
            builder = _build_program_pairz
        else:
            wmaps = _pair_weights(a, b, np.asarray(W_hh, np.float32),
                                  np.asarray(W_out, np.float32))
            builder = _build_program_pair
            # prescaled pair C'': tile r = [b*c_{2r+1} ; (b/a)*c_{2r}]
            bf = b.astype(np.float32)[None, :]
            baf = (b / a).astype(np.float32)[None, :]
    else:
        builder = _build_program

    if "nc" not in _cached:
        _cached["nc"] = builder()
    nc = _cached["nc"]

    in_maps = []
    for i in range(N_CORES):
        Cc = C[i * B:(i + 1) * B]                        # [B, S, H]
        if SCHEME == "v7":
            Cb, C2 = _v7_cc(Cc, a, b)
            in_maps.append({"in_Cb": Cb, "in_C2": C2, **wmaps})
            continue
        if SCHEME == "pairz":
            C_core = _pairz_cc(Cc, a, b)                 # [NPAIR, 2H, 2H]
        elif SCHEME == "pair":
            odd = (Cc[:, 1::2, :] * bf).transpose(1, 2, 0)   # [NPAIR, H, B]
            even = (Cc[:, 0::2, :] * baf).transpose(1, 2, 0)
            C_core = np.ascontiguousarray(
                np.concatenate([odd, even], axis=1))     # [NPAIR, 2H, B]
        else:
            C_core = np.ascontiguousarray(Cc.transpose(1, 2, 0))  # [S, H, B]
        in_maps.append({"in_C": C_core, **wmaps})

    core_ids = list(range(N_CORES))
    _cached["in_maps"] = in_maps
    res = run_bass_kernel_spmd(nc, in_maps, core_ids)

    out = np.empty((B_FULL, S, 1), dtype=np.float32)
    for i in range(N_CORES):
        if SCHEME == "v7":
            dev = res.results[i]["out"].reshape(NGRP, 2, GRP, B)
            out[i * B:(i + 1) * B, :, 0] = (
                dev.transpose(3, 0, 2, 1).reshape(B, S) + b_out[0])
            continue
        if SCHEME in ("pair", "pairz"):
            dev = res.results[i]["out"].reshape(NSEGP, 2, SEGP, B)
            dev = dev.transpose(0, 2, 1, 3).reshape(S, B)   # [o, b]
        else:
            dev = res.results[i]["out"].reshape(S, B)        # [s, b_local]
        out[i * B:(i + 1) * B, :, 0] = dev.T + b_out[0]
    return out


def _in_maps_for_test(C, wmaps):
    maps = []
    for i in range(N_CORES):
        C_core = np.ascontiguousarray(C[i * B:(i + 1) * B].transpose(1, 2, 0))
        maps.append({"in_C": C_core, **wmaps})
    return maps



# revision 19
# speedup vs baseline: 1.4244x; 1.3607x over previous
"""Trainium2 Bass kernel for nn_LiquidNeuralNetwork (B=512, S=1024, IN=16, HID=64).

Strategy
--------
The reference integrates dh/dt = (-h + tanh(h) @ W_hh.T + inp + bias) / tau
with RK4 x 4 substeps per timestep (16 sequential tanh+matmul rounds per
step).  At dt = 1/1023 the integration error of far cheaper schemes is orders
of magnitude below f32 rounding noise, so we integrate the same ODE with an
exponential integrator + AB2 extrapolation of the (tiny) tanh coupling term:

    H_s = a*H_{s-1} + b*(c_s + 1.5*g_{s-1} - 0.5*g_{s-2}),
    g_s = W_hh @ tanh(H_s),  a = exp(-dt/tau), b = 1 - a,
    c_s = W_ih @ (W_in x_s + b_in) + bias   (precomputed, hidden-major)

which agrees with the reference to ~6e-6 (the f32 noise floor of the
reference itself) while needing ONE tanh + matmul round per timestep.

On-device layout: hidden on partitions, batch on free dim; batch sharded
8 ways (64 per core).  Per round the PSUM bank accumulates the full affine
update via matmuls only:

    bank_r[0:64]  = diag(b) @ c_r + diag(a) @ hm_r + [Wp;Wm] @ [th_r;th_{r-1}]
    bank_r[64]    = W_out @ th_r          (the per-step scalar output)

with Wp = (1.5*b*W_hh)^T, Wm = (-0.5*b*W_hh)^T.  tanh runs on ACT straight
from PSUM; DVE copies bank->SBUF (h materialization + output-row collection);
everything except ACT->PE->ACT is off the critical path.
"""

import os
import numpy as np

import concourse.bacc as bacc
import concourse.tile as tile
from concourse import mybir
from concourse.bass_utils import run_bass_kernel_spmd

F32 = mybir.dt.float32
H = 64          # hidden
BIN = 16        # input features
B_FULL = 512
S = int(os.environ.get("LNN_S", "1024"))   # harness always runs 1024
N_CORES = 8
B = B_FULL // N_CORES   # 64 per-core batch
SEG = 128 if S % 128 == 0 else S           # output segment length (steps)
N_SEG = S // SEG

TRACE = bool(int(os.environ.get("LNN_TRACE", "0")))
SCHEME = os.environ.get("LNN_SCHEME", "v7")   # "v7" | "pair" | "pairz" | "e2"

GRP = 8                       # pair-slots per bulk output matmul (v7)
NGRP = (S // 2) // GRP

NPAIR = S // 2                 # pair rounds
SEGP = NPAIR if NPAIR <= 256 else 256   # pair-slots per output segment
NSEGP = NPAIR // SEGP

_cached = {}


def _build_program():
    """Build + compile the Bass program (same NEFF for all cores)."""
    nc = bacc.Bacc("TRN2", target_bir_lowering=False, debug=False)

    in_C = nc.dram_tensor("in_C", (S, H, B), F32, kind="ExternalInput").ap()
    in_Aev = nc.dram_tensor("in_Aev", (2 * H, H + 1), F32, kind="ExternalInput").ap()
    in_Aod = nc.dram_tensor("in_Aod", (2 * H, H + 1), F32, kind="ExternalInput").ap()
    in_Atl = nc.dram_tensor("in_Atl", (2 * H, H + 1), F32, kind="ExternalInput").ap()
    in_Db = nc.dram_tensor("in_Db", (H, H + 1), F32, kind="ExternalInput").ap()
    in_Da = nc.dram_tensor("in_Da", (H, H), F32, kind="ExternalInput").ap()
    out_dram = nc.dram_tensor("out", (N_SEG, SEG * B), F32, kind="ExternalOutput").ap()

    TANH = mybir.ActivationFunctionType.Tanh

    with tile.TileContext(nc) as tc:
        with (
            tc.tile_pool(name="wts", bufs=1) as wts,
            tc.tile_pool(name="thp", bufs=1) as thp,
            tc.tile_pool(name="osb", bufs=2) as osbp,
            tc.tile_pool(name="cp", bufs=10) as cp,
            tc.tile_pool(name="hmp", bufs=3) as hmp,
            tc.tile_pool(name="hbank", bufs=4, space="PSUM") as hbank,
        ):
            t_Aev = wts.tile([2 * H, H + 1], F32, tag="aev")
            t_Aod = wts.tile([2 * H, H + 1], F32, tag="aod")
            t_Atl = wts.tile([2 * H, H + 1], F32, tag="atl")
            t_Db = wts.tile([H, H + 1], F32, tag="db")
            t_Da = wts.tile([H, H], F32, tag="da")
            nc.sync.dma_start(out=t_Aev, in_=in_Aev)
            nc.sync.dma_start(out=t_Aod, in_=in_Aod)
            nc.sync.dma_start(out=t_Atl, in_=in_Atl)
            nc.sync.dma_start(out=t_Db, in_=in_Db)
            nc.sync.dma_start(out=t_Da, in_=in_Da)

            # persistent tanh tile: half0 = th of even rounds, half1 = odd
            t_th = thp.tile([2 * H, B], F32, tag="th")
            nc.vector.memset(t_th, 0.0)

            # output staging: only partition 64 is used; slot o at free
            # offset (o % SEG)*B.  Two tiles ping-pong across segments.
            t_osb = [osbp.tile([H + 1, SEG * B], F32, tag="osb", name=f"t_osb{i}")
                     for i in range(2)]

            prev_bank = None
            for r in range(1, S):
                t_c = cp.tile([H, B], F32, tag="c")
                nc.sync.dma_start(out=t_c, in_=in_C[r])

                bank = hbank.tile([H + 1, B], F32, tag="bank")
                last = r == 1
                # M4 first (start=True): clears rows 0..64 (col H of Db is 0)
                nc.tensor.matmul(bank, t_Db, t_c, start=True, stop=last)

                if r >= 2:
                    o = r - 2          # output index evacuated this round
                    seg, slot = divmod(o, SEG)
                    # evacuate prev bank's output row (lane-aligned copy)
                    nc.vector.tensor_copy(
                        t_osb[seg % 2][H:H + 1, slot * B:(slot + 1) * B],
                        prev_bank[H:H + 1, :],
                    )
                    if slot == SEG - 1:
                        nc.sync.dma_start(
                            out=out_dram[seg],
                            in_=t_osb[seg % 2][H:H + 1, :],
                        )
                    # h materialization for the decay term
                    t_hm = hmp.tile([H, B], F32, tag="hm")
                    nc.vector.tensor_copy(t_hm, prev_bank[:H, :])
                    # tanh straight from PSUM into this round's th half
                    half = r % 2
                    nc.scalar.activation(
                        t_th[half * H:(half + 1) * H, :], prev_bank[:H, :], TANH)
                    nc.tensor.matmul(bank[:H, :], t_Da, t_hm,
                                     start=False, stop=False)
                    t_A = t_Aev if r % 2 == 0 else t_Aod
                    nc.tensor.matmul(bank, t_A, t_th, start=False, stop=True)
                prev_bank = bank

            # tail: evacuate out_{S-2}; th_S = tanh(H_{S-1}); out_{S-1}
            o = S - 2
            seg, slot = divmod(o, SEG)
            nc.vector.tensor_copy(
                t_osb[seg % 2][H:H + 1, slot * B:(slot + 1) * B],
                prev_bank[H:H + 1, :],
            )
            half = S % 2
            nc.scalar.activation(
                t_th[half * H:(half + 1) * H, :], prev_bank[:H, :], TANH)
            tbank = hbank.tile([H + 1, B], F32, tag="bank")
            nc.tensor.matmul(tbank, t_Atl, t_th, start=True, stop=True)
            o = S - 1
            seg, slot = divmod(o, SEG)
            nc.vector.tensor_copy(
                t_osb[seg % 2][H:H + 1, slot * B:(slot + 1) * B],
                tbank[H:H + 1, :],
            )
            nc.sync.dma_start(out=out_dram[seg], in_=t_osb[seg % 2][H:H + 1, :])

    nc.compile()
    return nc


def _build_program_v7():
    """Pair scheme v7: no f32 matmuls, one bf16 tanh/round, bulk output.

    State per round r (2 timesteps): P = [H_s; H_{s+1}] and the prescaled
    decay copy Q = [a*H_{s+1}; a^2*H_{s+1}], both f32 PSUM [2H, B].

        P_r = Q_{r-1} + Cb_r + LT1 @ T1_r        (DVE base + 1 bf16 matmul)
        Q_r = a^2*Q_{r-1} + C2_r + LT12 @ T1_r   (DVE mul+add + 1 bf16 matmul)
        T1_r = tanh(P_{r-1})  (single bf16 ACT, written into a group buffer)

    Cb/C2 are host-premixed bf16 tiles DMA'd on the Sync and ACT hwdge
    queues respectively.  Outputs: T1 tiles accumulate in a [2H, GRP*B]
    group buffer; every GRP rounds one bulk matmul LO @ thbuf produces
    [2, GRP*B] in PSUM, evacuated by GpSimd and DMA'd out via swdge.
    """
    nc = bacc.Bacc("TRN2", target_bir_lowering=False, debug=False)

    BF16 = mybir.dt.bfloat16
    NP = NPAIR

    NCHUNK = NP // GRP
    in_Cb = nc.dram_tensor("in_Cb", (NCHUNK, 2 * H, GRP * B), BF16,
                           kind="ExternalInput").ap()
    in_C2 = nc.dram_tensor("in_C2", (NCHUNK, 2 * H, GRP * B), BF16,
                           kind="ExternalInput").ap()
    in_LT1 = nc.dram_tensor("in_LT1", (2 * H, 2 * H), BF16,
                            kind="ExternalInput").ap()
    in_LT12 = nc.dram_tensor("in_LT12", (2 * H, 2 * H), BF16,
                             kind="ExternalInput").ap()
    in_LO = nc.dram_tensor("in_LO", (2 * H, 2), BF16,
                           kind="ExternalInput").ap()
    in_a2 = nc.dram_tensor("in_a2", (2 * H, 1), F32, kind="ExternalInput").ap()
    out_dram = nc.dram_tensor("out", (NGRP, 2, GRP * B), F32,
                              kind="ExternalOutput").ap()

    TANH = mybir.ActivationFunctionType.Tanh

    with tile.TileContext(nc) as tc:
        with (
            tc.tile_pool(name="wts", bufs=1) as wts,
            tc.tile_pool(name="cbp", bufs=4) as cbp,
            tc.tile_pool(name="c2p", bufs=4) as c2p,
            tc.tile_pool(name="tmpp", bufs=3) as tmpp,
            tc.tile_pool(name="thb", bufs=2) as thbp,
            tc.tile_pool(name="osb", bufs=2) as osbp,
            tc.tile_pool(name="pP", bufs=3, space="PSUM") as pP,
            tc.tile_pool(name="pQ", bufs=3, space="PSUM") as pQ,
            tc.tile_pool(name="pO", bufs=2, space="PSUM") as pO,
        ):
            t_LT1 = wts.tile([2 * H, 2 * H], BF16, name="t_LT1")
            t_LT12 = wts.tile([2 * H, 2 * H], BF16, name="t_LT12")
            t_LO = wts.tile([2 * H, 2], BF16, name="t_LO")
            t_a2 = wts.tile([2 * H, 1], F32, name="t_a2")
            nc.sync.dma_start(out=t_LT1, in_=in_LT1)
            nc.sync.dma_start(out=t_LT12, in_=in_LT12)
            nc.sync.dma_start(out=t_LO, in_=in_LO)
            nc.sync.dma_start(out=t_a2, in_=in_a2)

            t_thb = [thbp.tile([2 * H, GRP * B], BF16, tag="thb",
                               name=f"t_thb{i}") for i in range(2)]

            # chunked c-tile DMA: one [2H, GRP*B] transfer per GRP rounds
            # per stream, both on the Sync hwdge queue; prefetch 2 chunks.
            cb_t, c2_t = {}, {}

            def fetch(k):
                if k >= NCHUNK:
                    return
                cb_t[k] = cbp.tile([2 * H, GRP * B], BF16, tag="cb",
                                   name=f"cb{k % 4}")
                nc.sync.dma_start(out=cb_t[k], in_=in_Cb[k])
                c2_t[k] = c2p.tile([2 * H, GRP * B], BF16, tag="c2",
                                   name=f"c2{k % 4}")
                nc.sync.dma_start(out=c2_t[k], in_=in_C2[k])

            for k in range(3):
                fetch(k)

            # PSUM has_written bits drive accumulate-vs-overwrite for
            # start=False matmuls and persist across NEFF executions; set
            # them deterministically with one start=True zero matmul per
            # P/Q bank so the DVE-written bases below are never clobbered.
            t_zmm = thbp.tile([2 * H, B], BF16, name="t_zmm")
            nc.vector.memset(t_zmm, 0.0)
            for i in range(3):
                Pd = pP.tile([2 * H, B], F32, tag="P", name=f"Pd{i}")
                nc.tensor.matmul(Pd, t_LT1, t_zmm, start=True, stop=True)
                Qd = pQ.tile([2 * H, B], F32, tag="Q", name=f"Qd{i}")
                nc.tensor.matmul(Qd, t_LT12, t_zmm, start=True, stop=True)

            # boot: P(0) = CbP[0] = [0; b*c1]; Q(0) = C2[0]; P(1) base =
            # CbP[1] = C2(0) + Cb(1) (host-premixed).  in_Cb carries CbP.
            P_prev = pP.tile([2 * H, B], F32, tag="P", name="P0")
            Q_prev = pQ.tile([2 * H, B], F32, tag="Q")
            nc.vector.tensor_copy(P_prev, cb_t[0][:, 0:B])
            nc.vector.tensor_copy(Q_prev, c2_t[0][:, 0:B])

            prev_tmp, prev_T1 = None, None
            for r in range(1, NP):
                g, slot = divmod(r - 1, GRP)
                ck, cs = divmod(r, GRP)
                t_c2 = c2_t[ck][:, cs * B:(cs + 1) * B]

                # single bf16 tanh straight from PSUM into the group buffer
                T1 = t_thb[g % 2][:, slot * B:(slot + 1) * B]
                nc.scalar.activation(T1, P_prev, TANH)
                if cs == 1:
                    fetch(ck + 2)

                # P(r) base: a^2*Q(r-2) + (C2(r-1)+Cb(r)), all old inputs --
                # never stalls ahead of the spine ops below
                P = pP.tile([2 * H, B], F32, tag="P", name=f"P{r % 3}")
                if prev_tmp is None:
                    nc.vector.tensor_copy(P, cb_t[ck][:, cs * B:(cs + 1) * B])
                else:
                    nc.vector.tensor_add(P, prev_tmp,
                                         cb_t[ck][:, cs * B:(cs + 1) * B])
                # Q spine on DVE: t_tmp = a^2*Q(r-1);  Q(r) = t_tmp + C2(r)
                t_tmp = tmpp.tile([2 * H, B], F32, tag="tmp")
                nc.vector.tensor_scalar_mul(t_tmp, Q_prev, t_a2)
                Q = pQ.tile([2 * H, B], F32, tag="Q")
                nc.vector.tensor_add(Q, t_tmp, t_c2)

                # couplings: P(r) += LT12@T1(r-1) (early, old tanh), then
                # Q(r) += LT12@T1(r) (spine), then P(r) += LT1@T1(r) (stop)
                if prev_T1 is not None:
                    nc.tensor.matmul(P, t_LT12, prev_T1, start=False,
                                     stop=False, skip_group_check=True)
                nc.tensor.matmul(Q, t_LT12, T1, start=False, stop=True,
                                 skip_group_check=True)
                nc.tensor.matmul(P, t_LT1, T1, start=False, stop=True,
                                 skip_group_check=True)

                if slot == GRP - 1:
                    ob = pO.tile([2, GRP * B], F32, tag="ob")
                    nc.tensor.matmul(ob, t_LO, t_thb[g % 2],
                                     start=True, stop=True)
                    t_os = osbp.tile([2, GRP * B], F32, tag="os")
                    nc.vector.tensor_copy(t_os, ob)
                    nc.gpsimd.dma_start(out=out_dram[g], in_=t_os)

                P_prev, Q_prev = P, Q
                prev_tmp, prev_T1 = t_tmp, T1

            # tail: T1(NP) completes the last group
            g, slot = NGRP - 1, GRP - 1
            T1 = t_thb[g % 2][:, slot * B:(slot + 1) * B]
            nc.scalar.activation(T1, P_prev, TANH)
            ob = pO.tile([2, GRP * B], F32, tag="ob")
            nc.tensor.matmul(ob, t_LO, t_thb[g % 2], start=True, stop=True)
            t_os = osbp.tile([2, GRP * B], F32, tag="os")
            nc.vector.tensor_copy(t_os, ob)
            nc.gpsimd.dma_start(out=out_dram[g], in_=t_os)

    nc.compile()
    return nc


def _v7_weights(a, b, W_hh, W_out):
    """LT1/LT12 as in pairz, plus bf16 LO and the a^2 decay vector."""
    import ml_dtypes
    wm = _pairz_weights(a, b, W_hh, W_out)
    return {
        "in_LT1": wm["in_LT1"],
        "in_LT12": wm["in_LT12"],
        "in_LO": wm["in_LO"].astype(ml_dtypes.bfloat16),
        "in_a2": wm["in_a2"],
    }


def _v7_cc(Cc, a, b):
    """Host Cb/C2 tiles [NPAIR, 2H, B] bf16 for one core (see _pairz_cc)."""
    import ml_dtypes
    ab = a * b
    ce = Cc[:, 0::2, :].astype(np.float64)    # c_{2r}   [B, NPAIR, H]
    co = Cc[:, 1::2, :].astype(np.float64)    # c_{2r+1}
    z = ab * ce + b * co
    Bc = Cc.shape[0]
    Cb = np.empty((NPAIR, 2 * H, Bc), np.float32)
    C2 = np.empty((NPAIR, 2 * H, Bc), np.float32)
    Cb[:, :H] = (b * ce).transpose(1, 2, 0)
    Cb[:, H:] = z.transpose(1, 2, 0)
    C2[:, :H] = (a * z).transpose(1, 2, 0)
    C2[:, H:] = (a * a * z).transpose(1, 2, 0)
    c1 = co[:, 0, :]                          # [B, H]
    Cb[0, :H] = 0.0
    Cb[0, H:] = (b * c1).T
    C2[0, :H] = (ab * c1).T
    C2[0, H:] = (a * ab * c1).T

    # premixed P-base stream: CbP(0) = Cb(0) (boot P(0) tile);
    # CbP(r) = C2(r-1) + Cb(r)  -> P(r) base = a^2*Q(r-2) + CbP(r)
    CbP = np.empty_like(Cb)
    CbP[0] = Cb[0]
    CbP[1:] = C2[:-1] + Cb[1:]

    def chunk(arr):
        return np.ascontiguousarray(
            arr.reshape(NPAIR // GRP, GRP, 2 * H, Bc)
            .transpose(0, 2, 1, 3)
            .reshape(NPAIR // GRP, 2 * H, GRP * Bc)
        ).astype(ml_dtypes.bfloat16)

    return chunk(CbP), chunk(C2)


def _build_program_pairz():
    """Pair scheme v6 ("zlite"): 2 steps per round, NO f32 matmuls on PE.

    One PSUM tile P_r [128,128] per round: cols 0:64 "bank" = [H_s; H_{s+1}],
    cols 64:128 "bank2" = [a*H_{s+1}; a^2*H_{s+1}] (pre-scaled decay copies,
    maintained so the next round's injections are lane-aligned DVE ops):

        bank_r   = bank2_{r-1} + Cb''_r + coupling(th)          (1 DVE add)
        bank2_r  = a^2*bank2_{r-1} + C2''_r + coupling2(th)     (mul + add)

    with all c-terms host-premixed into C''.  PE does only: 2 bf16 coupling
    matmuls (accumulating onto the DVE-written base via start=False) and the
    f32 output matvec.  tanh pair: bf16 (coupling) + f32 (out matvec).
    """
    nc = bacc.Bacc("TRN2", target_bir_lowering=False, debug=False)

    BF16 = mybir.dt.bfloat16

    in_C = nc.dram_tensor("in_C", (NPAIR, 2 * H, 2 * H), F32,
                          kind="ExternalInput").ap()
    ins = {}
    for nm in ("LT1", "LT12"):
        ins[nm] = nc.dram_tensor(f"in_{nm}", (2 * H, 2 * H), BF16,
                                 kind="ExternalInput").ap()
    ins["LO"] = nc.dram_tensor("in_LO", (2 * H, 2), F32,
                               kind="ExternalInput").ap()
    in_a2 = nc.dram_tensor("in_a2", (2 * H, 1), F32, kind="ExternalInput").ap()
    out_dram = nc.dram_tensor("out", (NSEGP, 2, SEGP * B), F32,
                              kind="ExternalOutput").ap()

    TANH = mybir.ActivationFunctionType.Tanh

    with tile.TileContext(nc) as tc:
        with (
            tc.tile_pool(name="wts", bufs=1) as wts,
            tc.tile_pool(name="thp", bufs=3) as thp,
            tc.tile_pool(name="thf", bufs=3) as thfp,
            tc.tile_pool(name="osb", bufs=2) as osbp,
            tc.tile_pool(name="ccp", bufs=6) as ccp,
            tc.tile_pool(name="tmpp", bufs=3) as tmpp,
            tc.tile_pool(name="pbank", bufs=4, space="PSUM") as pbank,
            tc.tile_pool(name="obank", bufs=3, space="PSUM") as obankp,
        ):
            t_w = {}
            for nm in ("LT1", "LT12"):
                t_w[nm] = wts.tile([2 * H, 2 * H], BF16, name=f"t_{nm}")
                nc.sync.dma_start(out=t_w[nm], in_=ins[nm])
            t_w["LO"] = wts.tile([2 * H, 2], F32, name="t_LO")
            nc.sync.dma_start(out=t_w["LO"], in_=ins["LO"])
            t_a2 = wts.tile([2 * H, 1], F32, name="t_a2")
            nc.sync.dma_start(out=t_a2, in_=in_a2)

            t_osb = [osbp.tile([2, SEGP * B], F32, tag="osb", name=f"t_osb{i}")
                     for i in range(2)]

            # boot: P_0 = C''_0 (H_0 = 0 so no decay/coupling terms)
            t_cc = ccp.tile([2 * H, 2 * H], F32, tag="cc")
            nc.sync.dma_start(out=t_cc, in_=in_C[0])
            P = pbank.tile([2 * H, 2 * H], F32, tag="P")
            nc.vector.tensor_copy(P, t_cc)

            prev_P = P
            prev_thf = None
            pending = []

            def flush_one():
                ob_t, m = pending.pop(0)
                seg, slot = divmod(m, SEGP)
                nc.vector.tensor_copy(
                    t_osb[seg % 2][0:2, slot * B:(slot + 1) * B], ob_t)
                if slot == SEGP - 1:
                    nc.sync.dma_start(out=out_dram[seg],
                                      in_=t_osb[seg % 2][0:2, :])

            for r in range(1, NPAIR):
                t_cc = ccp.tile([2 * H, 2 * H], F32, tag="cc")
                nc.sync.dma_start(out=t_cc, in_=in_C[r])

                P = pbank.tile([2 * H, 2 * H], F32, tag="P")
                # critical-path injection: bank base = bank2_prev + Cb''
                nc.vector.tensor_add(P[:, :2 * H - H], prev_P[:, H:H + H],
                                     t_cc[:, 0:H])
                # off-path: bank2 base = a^2*bank2_prev + C2''
                t_tmp = tmpp.tile([2 * H, H], F32, tag="tmp")
                nc.vector.tensor_scalar_mul(t_tmp, prev_P[:, H:H + H], t_a2)
                nc.vector.tensor_add(P[:, H:H + H], t_tmp, t_cc[:, H:H + H])

                # tanh pair from prev bank
                T1 = thp.tile([2 * H, B], BF16, tag="t1")
                nc.scalar.activation(T1, prev_P[:, 0:H], TANH)
                t_thf = thfp.tile([2 * H, B], F32, tag="thf")
                nc.scalar.activation(t_thf, prev_P[:, 0:H], TANH)

                # previous round's out matvec (f32) while ACT runs
                if prev_thf is not None:
                    ob = obankp.tile([2, B], F32, tag="ob")
                    nc.tensor.matmul(ob, t_w["LO"], prev_thf,
                                     start=True, stop=True)
                    pending.append((ob, r - 2))
                if len(pending) > 1:
                    flush_one()

                # coupling matmuls accumulate onto the DVE-written base
                nc.tensor.matmul(P[:, 0:H], t_w["LT1"], T1,
                                 start=False, stop=False,
                                 skip_group_check=True)
                nc.tensor.matmul(P[:, H:H + H], t_w["LT12"], T1,
                                 start=False, stop=True,
                                 skip_group_check=True)

                prev_P, prev_thf = P, t_thf

            # tail
            ob = obankp.tile([2, B], F32, tag="ob")
            nc.tensor.matmul(ob, t_w["LO"], prev_thf, start=True, stop=True)
            pending.append((ob, NPAIR - 2))
            t_thf = thfp.tile([2 * H, B], F32, tag="thf")
            nc.scalar.activation(t_thf, prev_P[:, 0:H], TANH)
            ob = obankp.tile([2, B], F32, tag="ob")
            nc.tensor.matmul(ob, t_w["LO"], t_thf, start=True, stop=True)
            pending.append((ob, NPAIR - 1))
            while pending:
                flush_one()

    nc.compile()
    return nc


def _pairz_weights(a, b, W_hh, W_out):
    """Host lhsT matrices + per-partition a^2 vector for the v6 scheme."""
    import ml_dtypes
    W = W_hh.astype(np.float64)
    wout = W_out[0].astype(np.float64)
    ab = a * b

    def blk(v):
        return (v[:, None] * W).T

    cp1, cm1 = 1.5 * b, -0.5 * b
    cp2, cm2 = 1.5 * ab + 2.5 * b, -0.5 * ab - 1.5 * b

    LT1 = np.zeros((2 * H, 2 * H))
    LT1[:H, :H] = blk(cm1)
    LT1[H:, :H] = blk(cp1)
    LT1[:H, H:] = blk(cm2)
    LT1[H:, H:] = blk(cp2)
    LT12 = np.zeros((2 * H, 2 * H))
    LT12[:H, :H] = blk(a * cm2)
    LT12[H:, :H] = blk(a * cp2)
    LT12[:H, H:] = blk(a * a * cm2)
    LT12[H:, H:] = blk(a * a * cp2)
    LO = np.zeros((2 * H, 2))
    LO[:H, 0] = wout
    LO[H:, 1] = wout
    a2v = np.concatenate([a * a, a * a]).reshape(2 * H, 1)
    return {"in_LT1": LT1.astype(ml_dtypes.bfloat16),
            "in_LT12": LT12.astype(ml_dtypes.bfloat16),
            "in_LO": LO.astype(np.float32),
            "in_a2": a2v.astype(np.float32)}


def _pairz_cc(Cc, a, b):
    """Host C'' quadrant tiles [NPAIR, 2H, 2H] for one core.

    Cc: [B, S, H] raw input-current.  Quadrants (rows x cols):
      [:, :H]  (bank col):  [b*c_s ; ab*c_s + b*c_{s+1}]
      [:, H:]  (bank2 col): [a^2 b*c_s + ab*c_{s+1} ; a^3 b*c_s + a^2 b*c_{s+1}]
    Boot tile (r=0, H_0=0): bank col = [0 ; b*c_1],
      bank2 col = [ab*c_1 ; a^2 b*c_1].
    """
    ab = a * b
    ce = Cc[:, 0::2, :].astype(np.float64)    # c_{2r}   [B, NPAIR, H]
    co = Cc[:, 1::2, :].astype(np.float64)    # c_{2r+1}
    out = np.empty((NPAIR, 2 * H, 2 * H), np.float32)
    # bank col
    out[:, :H, :H] = (b * ce).transpose(1, 2, 0)
    out[:, H:, :H] = (ab * ce + b * co).transpose(1, 2, 0)
    # bank2 col
    out[:, :H, H:] = (a * (ab * ce + b * co)).transpose(1, 2, 0)
    out[:, H:, H:] = (a * a * (ab * ce + b * co)).transpose(1, 2, 0)
    # boot overrides (c_0 unused, H_0 = 0)
    c1 = co[:, 0, :]                          # [B, H]
    out[0, :H, :H] = 0.0
    out[0, H:, :H] = (b * c1).T
    out[0, :H, H:] = (ab * c1).T
    out[0, H:, H:] = (a * ab * c1).T
    return out


def _build_program_pair():
    """Pair-corrected scheme v2: 2 timesteps per tanh round (S/2 rounds).

    PSUM bank halves = [H_s ; H_{s+1}^pred]; one bf16 ACT tanh covers both
    and feeds the (tiny) tanh-coupling matmuls LT1/LT2 in bf16; a second f32
    tanh feeds the f32 output matvec.  The c-injection is folded into the
    f32 decay matmul LH via a host-prescaled C'' tile DMA'd into the hm
    tile, whose lower half gets H_{s-1} added by one DVE op:
        hm = [b*c_{s+1} ; (b/a)*c_s + H_{s-1}]
        LH @ hm = [a*H_{s-1}+b*c_s ; a^2*H_{s-1}+ab*c_s+b*c_{s+1}]
    """
    nc = bacc.Bacc("TRN2", target_bir_lowering=False, debug=False)

    BF16 = mybir.dt.bfloat16
    GDT = BF16 if os.environ.get("LNN_GDT", "bf16") == "bf16" else F32

    in_C = nc.dram_tensor("in_C", (NPAIR, 2 * H, B), F32,
                          kind="ExternalInput").ap()
    ins = {}
    for nm in ("LH", "LB"):
        ins[nm] = nc.dram_tensor(f"in_{nm}", (2 * H, 2 * H), F32,
                                 kind="ExternalInput").ap()
    for nm in ("LT1", "LT2"):
        ins[nm] = nc.dram_tensor(f"in_{nm}", (2 * H, 2 * H), GDT,
                                 kind="ExternalInput").ap()
    ins["LO"] = nc.dram_tensor("in_LO", (2 * H, 2), F32,
                               kind="ExternalInput").ap()
    out_dram = nc.dram_tensor("out", (NSEGP, 2, SEGP * B), F32,
                              kind="ExternalOutput").ap()

    TANH = mybir.ActivationFunctionType.Tanh

    with tile.TileContext(nc) as tc:
        with (
            tc.tile_pool(name="wts", bufs=1) as wts,
            tc.tile_pool(name="thp", bufs=4) as thp,
            tc.tile_pool(name="thf", bufs=3) as thfp,
            tc.tile_pool(name="thz", bufs=1) as thz,
            tc.tile_pool(name="osb", bufs=2) as osbp,
            tc.tile_pool(name="hmp", bufs=8) as hmp,
            tc.tile_pool(name="hbank", bufs=4, space="PSUM") as hbank,
            tc.tile_pool(name="obank", bufs=3, space="PSUM") as obankp,
        ):
            t_w = {}
            for nm in ("LH", "LB"):
                t_w[nm] = wts.tile([2 * H, 2 * H], F32, name=f"t_{nm}")
                nc.sync.dma_start(out=t_w[nm], in_=ins[nm])
            for nm in ("LT1", "LT2"):
                t_w[nm] = wts.tile([2 * H, 2 * H], GDT, name=f"t_{nm}")
                nc.sync.dma_start(out=t_w[nm], in_=ins[nm])
            t_w["LO"] = wts.tile([2 * H, 2], F32, name="t_LO")
            nc.sync.dma_start(out=t_w["LO"], in_=ins["LO"])

            t_zero = thz.tile([2 * H, B], GDT, tag="t1zero")
            nc.vector.memset(t_zero, 0.0)
            t_osb = [osbp.tile([2, SEGP * B], F32, tag="osb", name=f"t_osb{i}")
                     for i in range(2)]

            # boot: bank_0 = [0 ; b*c_1]  (C''_0 half0 = b*c_1)
            t_hm = hmp.tile([2 * H, B], F32, tag="hm")
            nc.sync.dma_start(out=t_hm, in_=in_C[0])
            bank = hbank.tile([2 * H, B], F32, tag="bank")
            nc.tensor.matmul(bank, t_w["LB"], t_hm, start=True, stop=True)

            prev_bank = bank
            prev_T1 = t_zero
            prev_thf = None           # f32 tanh pair awaiting its out matvec
            pending = []              # [(ob_tile, slot_index)] not yet evac'd

            def flush_one():
                ob_t, m = pending.pop(0)
                seg, slot = divmod(m, SEGP)
                nc.vector.tensor_copy(
                    t_osb[seg % 2][0:2, slot * B:(slot + 1) * B], ob_t)
                if slot == SEGP - 1:
                    nc.sync.dma_start(out=out_dram[seg],
                                      in_=t_osb[seg % 2][0:2, :])

            for r in range(1, NPAIR):
                t_hm = hmp.tile([2 * H, B], F32, tag="hm")
                nc.sync.dma_start(out=t_hm, in_=in_C[r])

                bank = hbank.tile([2 * H, B], F32, tag="bank")
                # bf16 matmul first (FWL-friendly after last round's bf16 LT1)
                nc.tensor.matmul(bank, t_w["LT2"], prev_T1,
                                 start=True, stop=False)
                # the two f32 matmuls adjacent: previous round's out matvec,
                # then the decay+input injection
                if prev_thf is not None:
                    ob = obankp.tile([2, B], F32, tag="ob")
                    nc.tensor.matmul(ob, t_w["LO"], prev_thf,
                                     start=True, stop=True)
                    pending.append((ob, r - 2))

                # tanh pair: bf16 for the coupling path (critical), f32 for
                # the output matvec (off critical path)
                T1 = thp.tile([2 * H, B], GDT, tag="t1")
                nc.scalar.activation(T1, prev_bank, TANH)
                t_thf = thfp.tile([2 * H, B], F32, tag="thf")
                nc.scalar.activation(t_thf, prev_bank, TANH)

                # hm lower half += H_{s-1} (from prev bank)
                nc.vector.tensor_add(t_hm[H:, :], t_hm[H:, :],
                                     prev_bank[H:, :])

                if len(pending) > 1:
                    flush_one()

                nc.tensor.matmul(bank, t_w["LH"], t_hm, start=False,
                                 stop=False)
                nc.tensor.matmul(bank, t_w["LT1"], T1, start=False, stop=True)

                prev_bank, prev_T1, prev_thf = bank, T1, t_thf

            # tail: emit out matvecs for the last two tanh pairs, flush all
            ob = obankp.tile([2, B], F32, tag="ob")
            nc.tensor.matmul(ob, t_w["LO"], prev_thf, start=True, stop=True)
            pending.append((ob, NPAIR - 2))
            t_thf = thfp.tile([2 * H, B], F32, tag="thf")
            nc.scalar.activation(t_thf, prev_bank, TANH)
            ob = obankp.tile([2, B], F32, tag="ob")
            nc.tensor.matmul(ob, t_w["LO"], t_thf, start=True, stop=True)
            pending.append((ob, NPAIR - 1))
            while pending:
                flush_one()   # final segment's DMA fires on its last slot

    nc.compile()
    return nc



def _pair_weights(a, b, W_hh, W_out):
    """Host lhsT matrices for the pair-corrected scheme (f64 in)."""
    import ml_dtypes
    gdt = (ml_dtypes.bfloat16 if os.environ.get("LNN_GDT", "bf16") == "bf16"
           else np.float32)
    W = W_hh.astype(np.float64)
    wout = W_out[0].astype(np.float64)
    ab, a2, a2b = a * b, a * a, a * a * b

    def blk(v):
        return (v[:, None] * W).T

    LH = np.zeros((2 * H, 2 * H))
    LH[:H, H:] = np.eye(H)
    LH[H:, :H] = np.diag(a)
    LH[H:, H:] = np.diag(a2)
    LT1 = np.zeros((2 * H, 2 * H))
    LT1[:H, :H] = blk(-0.5 * b + 1.5 * ab)
    LT1[:H, H:] = blk(-0.5 * ab + 1.5 * a2b - 1.5 * b)
    LT1[H:, :H] = blk(1.5 * b)
    LT1[H:, H:] = blk(1.5 * ab + 2.5 * b)
    LT2 = np.zeros((2 * H, 2 * H))
    LT2[:H, :H] = blk(1.5 * ab)
    LT2[:H, H:] = blk(1.5 * a2b)
    LT2[H:, :H] = blk(-3.0 * ab)
    LT2[H:, H:] = blk(-3.0 * a2b)
    LB = np.zeros((2 * H, 2 * H))
    LB[:H, H:] = np.eye(H)
    LO = np.zeros((2 * H, 2))
    LO[:H, 0] = wout
    LO[H:, 1] = wout
    return {"in_LH": LH.astype(np.float32),
            "in_LB": LB.astype(np.float32),
            "in_LT1": LT1.astype(gdt),
            "in_LT2": LT2.astype(gdt),
            "in_LO": LO.astype(np.float32)}



def _host_precompute(x, W_in, b_in, W_hh, W_ih, bias, tau, W_out, b_out):
    x = np.asarray(x, dtype=np.float32)
    W_in = np.asarray(W_in, dtype=np.float32)
    b_in = np.asarray(b_in, dtype=np.float32)
    W_hh = np.asarray(W_hh, dtype=np.float32)
    W_ih = np.asarray(W_ih, dtype=np.float32)
    bias = np.asarray(bias, dtype=np.float32)
    tau = np.asarray(tau, dtype=np.float32)
    W_out = np.asarray(W_out, dtype=np.float32)

    W_comb = W_ih @ W_in                      # [H, BIN]
    b_comb = W_ih @ b_in + bias               # [H]
    C = x @ W_comb.T + b_comb                 # [B_FULL, S, H] f32

    t = np.linspace(0.0, 1.0, S).astype(np.float32)
    dt = np.float64(t[1]) - np.float64(t[0])
    d = 1.0 / tau.astype(np.float64)
    a = np.exp(-d * dt)
    b = 1.0 - a

    Wp = (1.5 * b[:, None] * W_hh.astype(np.float64)).T   # lhsT [k, j]
    Wm = (-0.5 * b[:, None] * W_hh.astype(np.float64)).T
    wout = W_out[0].astype(np.float64)                    # [H]

    Aev = np.zeros((2 * H, H + 1), np.float64)
    Aev[:H, :H] = Wp
    Aev[H:, :H] = Wm
    Aev[:H, H] = wout
    Aod = np.zeros((2 * H, H + 1), np.float64)
    Aod[:H, :H] = Wm
    Aod[H:, :H] = Wp
    Aod[H:, H] = wout
    # tail round index S (=1024, even): th_S lives in half S%2
    Atl = np.zeros((2 * H, H + 1), np.float64)
    if S % 2 == 0:
        Atl[:H, H] = wout
    else:
        Atl[H:, H] = wout
    Db = np.zeros((H, H + 1), np.float64)
    Db[:, :H] = np.diag(b)
    Da = np.diag(a)

    return C, {
        "in_Aev": Aev.astype(np.float32),
        "in_Aod": Aod.astype(np.float32),
        "in_Atl": Atl.astype(np.float32),
        "in_Db": Db.astype(np.float32),
        "in_Da": Da.astype(np.float32),
    }


def kernel(x, W_in, b_in, W_hh, W_ih, bias, tau, W_out, b_out):
    C, wmaps = _host_precompute(x, W_in, b_in, W_hh, W_ih, bias, tau,
                                W_out, b_out)
    b_out = np.asarray(b_out, dtype=np.float32)

    if SCHEME in ("pair", "pairz", "v7"):
        t = np.linspace(0.0, 1.0, S).astype(np.float32)
        dt = np.float64(t[1]) - np.float64(t[0])
        d = 1.0 / np.asarray(tau, dtype=np.float32).astype(np.float64)
        a = np.exp(-d * dt)
        b = 1.0 - a
        if SCHEME == "v7":
            wmaps = _v7_weights(a, b, np.asarray(W_hh, np.float32),
                                np.asarray(W_out, np.float32))
            builder = _build_program_v7
        elif SCHEME == "pairz":
            wmaps = _pairz_weights(a, b, np.asarray(W_hh, np.float32),
                                   np.asarray(W_out, np.float32))
            builder = _build_program_pairz
        else:
            wmaps = _pair_weights(a, b, np.asarray(W_hh, np.float32),
                                  np.asarray(W_out, np.float32))
            builder = _build_program_pair
            # prescaled pair C'': tile r = [b*c_{2r+1} ; (b/a)*c_{2r}]
            bf = b.astype(np.float32)[None, :]
            baf = (b / a).astype(np.float32)[None, :]
    else:
        builder = _build_program

    if "nc" not in _cached:
        _cached["nc"] = builder()
    nc = _cached["nc"]

    in_maps = []
    for i in range(N_CORES):
        Cc = C[i * B:(i + 1) * B]                        # [B, S, H]
        if SCHEME == "v7":
            Cb, C2 = _v7_cc(Cc, a, b)
            in_maps.append({"in_Cb": Cb, "in_C2": C2, **wmaps})
            continue
        if SCHEME == "pairz":
            C_core = _pairz_cc(Cc, a, b)                 # [NPAIR, 2H, 2H]
        elif SCHEME == "pair":
            odd = (Cc[:, 1::2, :] * bf).transpose(1, 2, 0)   # [NPAIR, H, B]
            even = (Cc[:, 0::2, :] * baf).transpose(1, 2, 0)
            C_core = np.ascontiguousarray(
                np.concatenate([odd, even], axis=1))     # [NPAIR, 2H, B]
        else:
            C_core = np.ascontiguousarray(Cc.transpose(1, 2, 0))  # [S, H, B]
        in_maps.append({"in_C": C_core, **wmaps})

    core_ids = list(range(N_CORES))
    _cached["in_maps"] = in_maps
    res = run_bass_kernel_spmd(nc, in_maps, core_ids)

    out = np.empty((B_FULL, S, 1), dtype=np.float32)
    for i in range(N_CORES):
        if SCHEME == "v7":
            dev = res.results[i]["out"].reshape(NGRP, 2, GRP, B)
            out[i * B:(i + 1) * B, :, 0] = (
                dev.transpose(3, 0, 2, 1).reshape(B, S) + b_out[0])
            continue
        if SCHEME in ("pair", "pairz"):
            dev = res.results[i]["out"].reshape(NSEGP, 2, SEGP, B)
            dev = dev.transpose(0, 2, 1, 3).reshape(S, B)   # [o, b]
        else:
            dev = res.results[i]["out"].reshape(S, B)        # [s, b_local]
        out[i * B:(i + 1) * B, :, 0] = dev.T + b_out[0]
    return out


def _in_maps_for_test(C, wmaps):
    maps = []
    for i in range(N_CORES):
        C_core = np.ascontiguousarray(C[i * B:(i + 1) * B].transpose(1, 2, 0))
        maps.append({"in_C": C_core, **wmaps})
    return maps



# revision 20
# speedup vs baseline: 1.5521x; 1.0897x over previous
"""Trainium2 Bass kernel for nn_LiquidNeuralNetwork (B=512, S=1024, IN=16, HID=64).

Strategy
--------
The reference integrates dh/dt = (-h + tanh(h) @ W_hh.T + inp + bias) / tau
with RK4 x 4 substeps per timestep (16 sequential tanh+matmul rounds per
step).  At dt = 1/1023 the integration error of far cheaper schemes is orders
of magnitude below f32 rounding noise, so we integrate the same ODE with an
exponential integrator + AB2 extrapolation of the (tiny) tanh coupling term:

    H_s = a*H_{s-1} + b*(c_s + 1.5*g_{s-1} - 0.5*g_{s-2}),
    g_s = W_hh @ tanh(H_s),  a = exp(-dt/tau), b = 1 - a,
    c_s = W_ih @ (W_in x_s + b_in) + bias   (precomputed, hidden-major)

which agrees with the reference to ~6e-6 (the f32 noise floor of the
reference itself) while needing ONE tanh + matmul round per timestep.

On-device layout: hidden on partitions, batch on free dim; batch sharded
8 ways (64 per core).  Per round the PSUM bank accumulates the full affine
update via matmuls only:

    bank_r[0:64]  = diag(b) @ c_r + diag(a) @ hm_r + [Wp;Wm] @ [th_r;th_{r-1}]
    bank_r[64]    = W_out @ th_r          (the per-step scalar output)

with Wp = (1.5*b*W_hh)^T, Wm = (-0.5*b*W_hh)^T.  tanh runs on ACT straight
from PSUM; DVE copies bank->SBUF (h materialization + output-row collection);
everything except ACT->PE->ACT is off the critical path.
"""

import os
import numpy as np

import concourse.bacc as bacc
import concourse.tile as tile
from concourse import mybir
from concourse.bass_utils import run_bass_kernel_spmd

F32 = mybir.dt.float32
H = 64          # hidden
BIN = 16        # input features
B_FULL = 512
S = int(os.environ.get("LNN_S", "1024"))   # harness always runs 1024
N_CORES = 8
B = B_FULL // N_CORES   # 64 per-core batch
SEG = 128 if S % 128 == 0 else S           # output segment length (steps)
N_SEG = S // SEG

TRACE = bool(int(os.environ.get("LNN_TRACE", "0")))
SCHEME = os.environ.get("LNN_SCHEME", "v7")   # "v7" | "pair" | "pairz" | "e2"

GRP = 8                       # pair-slots per bulk output matmul (v7)
NGRP = (S // 2) // GRP

NPAIR = S // 2                 # pair rounds
SEGP = NPAIR if NPAIR <= 256 else 256   # pair-slots per output segment
NSEGP = NPAIR // SEGP

_cached = {}


def _build_program():
    """Build + compile the Bass program (same NEFF for all cores)."""
    nc = bacc.Bacc("TRN2", target_bir_lowering=False, debug=False)

    in_C = nc.dram_tensor("in_C", (S, H, B), F32, kind="ExternalInput").ap()
    in_Aev = nc.dram_tensor("in_Aev", (2 * H, H + 1), F32, kind="ExternalInput").ap()
    in_Aod = nc.dram_tensor("in_Aod", (2 * H, H + 1), F32, kind="ExternalInput").ap()
    in_Atl = nc.dram_tensor("in_Atl", (2 * H, H + 1), F32, kind="ExternalInput").ap()
    in_Db = nc.dram_tensor("in_Db", (H, H + 1), F32, kind="ExternalInput").ap()
    in_Da = nc.dram_tensor("in_Da", (H, H), F32, kind="ExternalInput").ap()
    out_dram = nc.dram_tensor("out", (N_SEG, SEG * B), F32, kind="ExternalOutput").ap()

    TANH = mybir.ActivationFunctionType.Tanh

    with tile.TileContext(nc) as tc:
        with (
            tc.tile_pool(name="wts", bufs=1) as wts,
            tc.tile_pool(name="thp", bufs=1) as thp,
            tc.tile_pool(name="osb", bufs=2) as osbp,
            tc.tile_pool(name="cp", bufs=10) as cp,
            tc.tile_pool(name="hmp", bufs=3) as hmp,
            tc.tile_pool(name="hbank", bufs=4, space="PSUM") as hbank,
        ):
            t_Aev = wts.tile([2 * H, H + 1], F32, tag="aev")
            t_Aod = wts.tile([2 * H, H + 1], F32, tag="aod")
            t_Atl = wts.tile([2 * H, H + 1], F32, tag="atl")
            t_Db = wts.tile([H, H + 1], F32, tag="db")
            t_Da = wts.tile([H, H], F32, tag="da")
            nc.sync.dma_start(out=t_Aev, in_=in_Aev)
            nc.sync.dma_start(out=t_Aod, in_=in_Aod)
            nc.sync.dma_start(out=t_Atl, in_=in_Atl)
            nc.sync.dma_start(out=t_Db, in_=in_Db)
            nc.sync.dma_start(out=t_Da, in_=in_Da)

            # persistent tanh tile: half0 = th of even rounds, half1 = odd
            t_th = thp.tile([2 * H, B], F32, tag="th")
            nc.vector.memset(t_th, 0.0)

            # output staging: only partition 64 is used; slot o at free
            # offset (o % SEG)*B.  Two tiles ping-pong across segments.
            t_osb = [osbp.tile([H + 1, SEG * B], F32, tag="osb", name=f"t_osb{i}")
                     for i in range(2)]

            prev_bank = None
            for r in range(1, S):
                t_c = cp.tile([H, B], F32, tag="c")
                nc.sync.dma_start(out=t_c, in_=in_C[r])

                bank = hbank.tile([H + 1, B], F32, tag="bank")
                last = r == 1
                # M4 first (start=True): clears rows 0..64 (col H of Db is 0)
                nc.tensor.matmul(bank, t_Db, t_c, start=True, stop=last)

                if r >= 2:
                    o = r - 2          # output index evacuated this round
                    seg, slot = divmod(o, SEG)
                    # evacuate prev bank's output row (lane-aligned copy)
                    nc.vector.tensor_copy(
                        t_osb[seg % 2][H:H + 1, slot * B:(slot + 1) * B],
                        prev_bank[H:H + 1, :],
                    )
                    if slot == SEG - 1:
                        nc.sync.dma_start(
                            out=out_dram[seg],
                            in_=t_osb[seg % 2][H:H + 1, :],
                        )
                    # h materialization for the decay term
                    t_hm = hmp.tile([H, B], F32, tag="hm")
                    nc.vector.tensor_copy(t_hm, prev_bank[:H, :])
                    # tanh straight from PSUM into this round's th half
                    half = r % 2
                    nc.scalar.activation(
                        t_th[half * H:(half + 1) * H, :], prev_bank[:H, :], TANH)
                    nc.tensor.matmul(bank[:H, :], t_Da, t_hm,
                                     start=False, stop=False)
                    t_A = t_Aev if r % 2 == 0 else t_Aod
                    nc.tensor.matmul(bank, t_A, t_th, start=False, stop=True)
                prev_bank = bank

            # tail: evacuate out_{S-2}; th_S = tanh(H_{S-1}); out_{S-1}
            o = S - 2
            seg, slot = divmod(o, SEG)
            nc.vector.tensor_copy(
                t_osb[seg % 2][H:H + 1, slot * B:(slot + 1) * B],
                prev_bank[H:H + 1, :],
            )
            half = S % 2
            nc.scalar.activation(
                t_th[half * H:(half + 1) * H, :], prev_bank[:H, :], TANH)
            tbank = hbank.tile([H + 1, B], F32, tag="bank")
            nc.tensor.matmul(tbank, t_Atl, t_th, start=True, stop=True)
            o = S - 1
            seg, slot = divmod(o, SEG)
            nc.vector.tensor_copy(
                t_osb[seg % 2][H:H + 1, slot * B:(slot + 1) * B],
                tbank[H:H + 1, :],
            )
            nc.sync.dma_start(out=out_dram[seg], in_=t_osb[seg % 2][H:H + 1, :])

    nc.compile()
    return nc


def _build_program_v7(a2_imm=None):
    """Pair scheme v7: no f32 matmuls, one bf16 tanh/round, bulk output.

    State per round r (2 timesteps): P = [H_s; H_{s+1}] and the prescaled
    decay copy Q = [a*H_{s+1}; a^2*H_{s+1}], both f32 PSUM [2H, B].

        P_r = Q_{r-1} + Cb_r + LT1 @ T1_r        (DVE base + 1 bf16 matmul)
        Q_r = a^2*Q_{r-1} + C2_r + LT12 @ T1_r   (DVE mul+add + 1 bf16 matmul)
        T1_r = tanh(P_{r-1})  (single bf16 ACT, written into a group buffer)

    Cb/C2 are host-premixed bf16 tiles DMA'd on the Sync and ACT hwdge
    queues respectively.  Outputs: T1 tiles accumulate in a [2H, GRP*B]
    group buffer; every GRP rounds one bulk matmul LO @ thbuf produces
    [2, GRP*B] in PSUM, evacuated by GpSimd and DMA'd out via swdge.
    """
    nc = bacc.Bacc("TRN2", target_bir_lowering=False, debug=False)

    BF16 = mybir.dt.bfloat16
    NP = NPAIR

    NCHUNK = NP // GRP
    in_Cb = nc.dram_tensor("in_Cb", (NCHUNK, 2 * H, GRP * B), BF16,
                           kind="ExternalInput").ap()
    in_C2 = nc.dram_tensor("in_C2", (NCHUNK, 2 * H, GRP * B), BF16,
                           kind="ExternalInput").ap()
    in_LT1 = nc.dram_tensor("in_LT1", (2 * H, 2 * H), BF16,
                            kind="ExternalInput").ap()
    in_LT12 = nc.dram_tensor("in_LT12", (2 * H, 2 * H), BF16,
                             kind="ExternalInput").ap()
    in_LO = nc.dram_tensor("in_LO", (2 * H, 2), BF16,
                           kind="ExternalInput").ap()
    in_a2 = nc.dram_tensor("in_a2", (2 * H, 1), F32, kind="ExternalInput").ap()
    out_dram = nc.dram_tensor("out", (NGRP, 2, GRP * B), F32,
                              kind="ExternalOutput").ap()

    TANH = mybir.ActivationFunctionType.Tanh

    with tile.TileContext(nc) as tc:
        with (
            tc.tile_pool(name="wts", bufs=1) as wts,
            tc.tile_pool(name="cbp", bufs=4) as cbp,
            tc.tile_pool(name="c2p", bufs=4) as c2p,
            tc.tile_pool(name="tmpp", bufs=3) as tmpp,
            tc.tile_pool(name="thb", bufs=2) as thbp,
            tc.tile_pool(name="osb", bufs=2) as osbp,
            tc.tile_pool(name="pP", bufs=3, space="PSUM") as pP,
            tc.tile_pool(name="pQ", bufs=3, space="PSUM") as pQ,
            tc.tile_pool(name="pO", bufs=2, space="PSUM") as pO,
        ):
            t_LT1 = wts.tile([2 * H, 2 * H], BF16, name="t_LT1")
            t_LT12 = wts.tile([2 * H, 2 * H], BF16, name="t_LT12")
            t_LO = wts.tile([2 * H, 2], BF16, name="t_LO")
            t_a2 = wts.tile([2 * H, 1], F32, name="t_a2")
            nc.sync.dma_start(out=t_LT1, in_=in_LT1)
            nc.sync.dma_start(out=t_LT12, in_=in_LT12)
            nc.sync.dma_start(out=t_LO, in_=in_LO)
            nc.sync.dma_start(out=t_a2, in_=in_a2)

            t_thb = [thbp.tile([2 * H, GRP * B], BF16, tag="thb",
                               name=f"t_thb{i}") for i in range(2)]

            # chunked c-tile DMA: one [2H, GRP*B] transfer per GRP rounds
            # per stream, both on the Sync hwdge queue; prefetch 2 chunks.
            cb_t, c2_t = {}, {}

            def fetch(k):
                if k >= NCHUNK:
                    return
                cb_t[k] = cbp.tile([2 * H, GRP * B], BF16, tag="cb",
                                   name=f"cb{k % 4}")
                nc.sync.dma_start(out=cb_t[k], in_=in_Cb[k])
                c2_t[k] = c2p.tile([2 * H, GRP * B], BF16, tag="c2",
                                   name=f"c2{k % 4}")
                nc.sync.dma_start(out=c2_t[k], in_=in_C2[k])

            for k in range(3):
                fetch(k)

            # PSUM has_written bits drive accumulate-vs-overwrite for
            # start=False matmuls and persist across NEFF executions; set
            # them deterministically with one start=True zero matmul per
            # P/Q bank so the DVE-written bases below are never clobbered.
            t_zmm = thbp.tile([2 * H, B], BF16, name="t_zmm")
            nc.vector.memset(t_zmm, 0.0)
            for i in range(3):
                Pd = pP.tile([2 * H, B], F32, tag="P", name=f"Pd{i}")
                nc.tensor.matmul(Pd, t_LT1, t_zmm, start=True, stop=True)
                Qd = pQ.tile([2 * H, B], F32, tag="Q", name=f"Qd{i}")
                nc.tensor.matmul(Qd, t_LT12, t_zmm, start=True, stop=True)

            # boot: P(0) = CbP[0] = [0; b*c1]; Q(0) = C2[0]; P(1) base =
            # CbP[1] = C2(0) + Cb(1) (host-premixed).  in_Cb carries CbP.
            P_prev = pP.tile([2 * H, B], F32, tag="P", name="P0")
            Q_prev = pQ.tile([2 * H, B], F32, tag="Q")
            nc.vector.tensor_copy(P_prev, cb_t[0][:, 0:B])
            nc.vector.tensor_copy(Q_prev, c2_t[0][:, 0:B])

            prev_tmp, prev_T1 = None, None
            for r in range(1, NP):
                g, slot = divmod(r - 1, GRP)
                ck, cs = divmod(r, GRP)
                t_c2 = c2_t[ck][:, cs * B:(cs + 1) * B]

                # single bf16 tanh straight from PSUM into the group buffer
                T1 = t_thb[g % 2][:, slot * B:(slot + 1) * B]
                nc.scalar.activation(T1, P_prev, TANH)
                if cs == 1:
                    fetch(ck + 2)

                # P(r) base: a^2*Q(r-2) + (C2(r-1)+Cb(r)), all old inputs --
                # never stalls ahead of the spine ops below
                P = pP.tile([2 * H, B], F32, tag="P", name=f"P{r % 3}")
                if prev_tmp is None:
                    nc.vector.tensor_copy(P, cb_t[ck][:, cs * B:(cs + 1) * B])
                else:
                    nc.vector.tensor_add(P, prev_tmp,
                                         cb_t[ck][:, cs * B:(cs + 1) * B])
                # Q spine on DVE: t_tmp = a^2*Q(r-1);  Q(r) = t_tmp + C2(r)
                t_tmp = tmpp.tile([2 * H, B], F32, tag="tmp")
                nc.vector.tensor_scalar_mul(
                    t_tmp, Q_prev, a2_imm if a2_imm is not None else t_a2)
                Q = pQ.tile([2 * H, B], F32, tag="Q")
                nc.vector.tensor_add(Q, t_tmp, t_c2)

                # couplings: P(r) += LT12@T1(r-1) (early, old tanh), then
                # Q(r) += LT12@T1(r) (spine), then P(r) += LT1@T1(r) (stop)
                if prev_T1 is not None:
                    nc.tensor.matmul(P, t_LT12, prev_T1, start=False,
                                     stop=False, skip_group_check=True)
                nc.tensor.matmul(Q, t_LT12, T1, start=False, stop=True,
                                 skip_group_check=True)
                nc.tensor.matmul(P, t_LT1, T1, start=False, stop=True,
                                 skip_group_check=True)

                if slot == GRP // 2 - 1:
                    ob = pO.tile([2, GRP * B], F32, tag="ob")
                    ob_pend = ob
                    nc.tensor.matmul(ob[:, :GRP * B // 2], t_LO,
                                     t_thb[g % 2][:, :GRP * B // 2],
                                     start=True, stop=False,
                                     skip_group_check=True)
                elif slot == GRP - 1:
                    ob = ob_pend
                    nc.tensor.matmul(ob[:, GRP * B // 2:], t_LO,
                                     t_thb[g % 2][:, GRP * B // 2:],
                                     start=False, stop=True,
                                     skip_group_check=True)
                    t_os = osbp.tile([2, GRP * B], F32, tag="os")
                    nc.scalar.copy(t_os, ob)
                    nc.gpsimd.dma_start(out=out_dram[g], in_=t_os)

                P_prev, Q_prev = P, Q
                prev_tmp, prev_T1 = t_tmp, T1

            # tail: T1(NP) completes the last group
            g, slot = NGRP - 1, GRP - 1
            T1 = t_thb[g % 2][:, slot * B:(slot + 1) * B]
            nc.scalar.activation(T1, P_prev, TANH)
            ob = ob_pend
            nc.tensor.matmul(ob[:, GRP * B // 2:], t_LO,
                             t_thb[g % 2][:, GRP * B // 2:],
                             start=False, stop=True, skip_group_check=True)
            t_os = osbp.tile([2, GRP * B], F32, tag="os")
            nc.scalar.copy(t_os, ob)
            nc.gpsimd.dma_start(out=out_dram[g], in_=t_os)

    nc.compile()
    return nc


def _v7_weights(a, b, W_hh, W_out):
    """LT1/LT12 as in pairz, plus bf16 LO and the a^2 decay vector."""
    import ml_dtypes
    wm = _pairz_weights(a, b, W_hh, W_out)
    return {
        "in_LT1": wm["in_LT1"],
        "in_LT12": wm["in_LT12"],
        "in_LO": wm["in_LO"].astype(ml_dtypes.bfloat16),
        "in_a2": wm["in_a2"],
    }


def _v7_cc(Cc, a, b):
    """Host Cb/C2 tiles [NPAIR, 2H, B] bf16 for one core (see _pairz_cc)."""
    import ml_dtypes
    ab = a * b
    ce = Cc[:, 0::2, :].astype(np.float64)    # c_{2r}   [B, NPAIR, H]
    co = Cc[:, 1::2, :].astype(np.float64)    # c_{2r+1}
    z = ab * ce + b * co
    Bc = Cc.shape[0]
    Cb = np.empty((NPAIR, 2 * H, Bc), np.float32)
    C2 = np.empty((NPAIR, 2 * H, Bc), np.float32)
    Cb[:, :H] = (b * ce).transpose(1, 2, 0)
    Cb[:, H:] = z.transpose(1, 2, 0)
    C2[:, :H] = (a * z).transpose(1, 2, 0)
    C2[:, H:] = (a * a * z).transpose(1, 2, 0)
    c1 = co[:, 0, :]                          # [B, H]
    Cb[0, :H] = 0.0
    Cb[0, H:] = (b * c1).T
    C2[0, :H] = (ab * c1).T
    C2[0, H:] = (a * ab * c1).T

    # premixed P-base stream: CbP(0) = Cb(0) (boot P(0) tile);
    # CbP(r) = C2(r-1) + Cb(r)  -> P(r) base = a^2*Q(r-2) + CbP(r)
    CbP = np.empty_like(Cb)
    CbP[0] = Cb[0]
    CbP[1:] = C2[:-1] + Cb[1:]

    def chunk(arr):
        return np.ascontiguousarray(
            arr.reshape(NPAIR // GRP, GRP, 2 * H, Bc)
            .transpose(0, 2, 1, 3)
            .reshape(NPAIR // GRP, 2 * H, GRP * Bc)
        ).astype(ml_dtypes.bfloat16)

    return chunk(CbP), chunk(C2)


def _build_program_pairz():
    """Pair scheme v6 ("zlite"): 2 steps per round, NO f32 matmuls on PE.

    One PSUM tile P_r [128,128] per round: cols 0:64 "bank" = [H_s; H_{s+1}],
    cols 64:128 "bank2" = [a*H_{s+1}; a^2*H_{s+1}] (pre-scaled decay copies,
    maintained so the next round's injections are lane-aligned DVE ops):

        bank_r   = bank2_{r-1} + Cb''_r + coupling(th)          (1 DVE add)
        bank2_r  = a^2*bank2_{r-1} + C2''_r + coupling2(th)     (mul + add)

    with all c-terms host-premixed into C''.  PE does only: 2 bf16 coupling
    matmuls (accumulating onto the DVE-written base via start=False) and the
    f32 output matvec.  tanh pair: bf16 (coupling) + f32 (out matvec).
    """
    nc = bacc.Bacc("TRN2", target_bir_lowering=False, debug=False)

    BF16 = mybir.dt.bfloat16

    in_C = nc.dram_tensor("in_C", (NPAIR, 2 * H, 2 * H), F32,
                          kind="ExternalInput").ap()
    ins = {}
    for nm in ("LT1", "LT12"):
        ins[nm] = nc.dram_tensor(f"in_{nm}", (2 * H, 2 * H), BF16,
                                 kind="ExternalInput").ap()
    ins["LO"] = nc.dram_tensor("in_LO", (2 * H, 2), F32,
                               kind="ExternalInput").ap()
    in_a2 = nc.dram_tensor("in_a2", (2 * H, 1), F32, kind="ExternalInput").ap()
    out_dram = nc.dram_tensor("out", (NSEGP, 2, SEGP * B), F32,
                              kind="ExternalOutput").ap()

    TANH = mybir.ActivationFunctionType.Tanh

    with tile.TileContext(nc) as tc:
        with (
            tc.tile_pool(name="wts", bufs=1) as wts,
            tc.tile_pool(name="thp", bufs=3) as thp,
            tc.tile_pool(name="thf", bufs=3) as thfp,
            tc.tile_pool(name="osb", bufs=2) as osbp,
            tc.tile_pool(name="ccp", bufs=6) as ccp,
            tc.tile_pool(name="tmpp", bufs=3) as tmpp,
            tc.tile_pool(name="pbank", bufs=4, space="PSUM") as pbank,
            tc.tile_pool(name="obank", bufs=3, space="PSUM") as obankp,
        ):
            t_w = {}
            for nm in ("LT1", "LT12"):
                t_w[nm] = wts.tile([2 * H, 2 * H], BF16, name=f"t_{nm}")
                nc.sync.dma_start(out=t_w[nm], in_=ins[nm])
            t_w["LO"] = wts.tile([2 * H, 2], F32, name="t_LO")
            nc.sync.dma_start(out=t_w["LO"], in_=ins["LO"])
            t_a2 = wts.tile([2 * H, 1], F32, name="t_a2")
            nc.sync.dma_start(out=t_a2, in_=in_a2)

            t_osb = [osbp.tile([2, SEGP * B], F32, tag="osb", name=f"t_osb{i}")
                     for i in range(2)]

            # boot: P_0 = C''_0 (H_0 = 0 so no decay/coupling terms)
            t_cc = ccp.tile([2 * H, 2 * H], F32, tag="cc")
            nc.sync.dma_start(out=t_cc, in_=in_C[0])
            P = pbank.tile([2 * H, 2 * H], F32, tag="P")
            nc.vector.tensor_copy(P, t_cc)

            prev_P = P
            prev_thf = None
            pending = []

            def flush_one():
                ob_t, m = pending.pop(0)
                seg, slot = divmod(m, SEGP)
                nc.vector.tensor_copy(
                    t_osb[seg % 2][0:2, slot * B:(slot + 1) * B], ob_t)
                if slot == SEGP - 1:
                    nc.sync.dma_start(out=out_dram[seg],
                                      in_=t_osb[seg % 2][0:2, :])

            for r in range(1, NPAIR):
                t_cc = ccp.tile([2 * H, 2 * H], F32, tag="cc")
                nc.sync.dma_start(out=t_cc, in_=in_C[r])

                P = pbank.tile([2 * H, 2 * H], F32, tag="P")
                # critical-path injection: bank base = bank2_prev + Cb''
                nc.vector.tensor_add(P[:, :2 * H - H], prev_P[:, H:H + H],
                                     t_cc[:, 0:H])
                # off-path: bank2 base = a^2*bank2_prev + C2''
                t_tmp = tmpp.tile([2 * H, H], F32, tag="tmp")
                nc.vector.tensor_scalar_mul(t_tmp, prev_P[:, H:H + H], t_a2)
                nc.vector.tensor_add(P[:, H:H + H], t_tmp, t_cc[:, H:H + H])

                # tanh pair from prev bank
                T1 = thp.tile([2 * H, B], BF16, tag="t1")
                nc.scalar.activation(T1, prev_P[:, 0:H], TANH)
                t_thf = thfp.tile([2 * H, B], F32, tag="thf")
                nc.scalar.activation(t_thf, prev_P[:, 0:H], TANH)

                # previous round's out matvec (f32) while ACT runs
                if prev_thf is not None:
                    ob = obankp.tile([2, B], F32, tag="ob")
                    nc.tensor.matmul(ob, t_w["LO"], prev_thf,
                                     start=True, stop=True)
                    pending.append((ob, r - 2))
                if len(pending) > 1:
                    flush_one()

                # coupling matmuls accumulate onto the DVE-written base
                nc.tensor.matmul(P[:, 0:H], t_w["LT1"], T1,
                                 start=False, stop=False,
                                 skip_group_check=True)
                nc.tensor.matmul(P[:, H:H + H], t_w["LT12"], T1,
                                 start=False, stop=True,
                                 skip_group_check=True)

                prev_P, prev_thf = P, t_thf

            # tail
            ob = obankp.tile([2, B], F32, tag="ob")
            nc.tensor.matmul(ob, t_w["LO"], prev_thf, start=True, stop=True)
            pending.append((ob, NPAIR - 2))
            t_thf = thfp.tile([2 * H, B], F32, tag="thf")
            nc.scalar.activation(t_thf, prev_P[:, 0:H], TANH)
            ob = obankp.tile([2, B], F32, tag="ob")
            nc.tensor.matmul(ob, t_w["LO"], t_thf, start=True, stop=True)
            pending.append((ob, NPAIR - 1))
            while pending:
                flush_one()

    nc.compile()
    return nc


def _pairz_weights(a, b, W_hh, W_out):
    """Host lhsT matrices + per-partition a^2 vector for the v6 scheme."""
    import ml_dtypes
    W = W_hh.astype(np.float64)
    wout = W_out[0].astype(np.float64)
    ab = a * b

    def blk(v):
        return (v[:, None] * W).T

    cp1, cm1 = 1.5 * b, -0.5 * b
    cp2, cm2 = 1.5 * ab + 2.5 * b, -0.5 * ab - 1.5 * b

    LT1 = np.zeros((2 * H, 2 * H))
    LT1[:H, :H] = blk(cm1)
    LT1[H:, :H] = blk(cp1)
    LT1[:H, H:] = blk(cm2)
    LT1[H:, H:] = blk(cp2)
    LT12 = np.zeros((2 * H, 2 * H))
    LT12[:H, :H] = blk(a * cm2)
    LT12[H:, :H] = blk(a * cp2)
    LT12[:H, H:] = blk(a * a * cm2)
    LT12[H:, H:] = blk(a * a * cp2)
    LO = np.zeros((2 * H, 2))
    LO[:H, 0] = wout
    LO[H:, 1] = wout
    a2v = np.concatenate([a * a, a * a]).reshape(2 * H, 1)
    return {"in_LT1": LT1.astype(ml_dtypes.bfloat16),
            "in_LT12": LT12.astype(ml_dtypes.bfloat16),
            "in_LO": LO.astype(np.float32),
            "in_a2": a2v.astype(np.float32)}


def _pairz_cc(Cc, a, b):
    """Host C'' quadrant tiles [NPAIR, 2H, 2H] for one core.

    Cc: [B, S, H] raw input-current.  Quadrants (rows x cols):
      [:, :H]  (bank col):  [b*c_s ; ab*c_s + b*c_{s+1}]
      [:, H:]  (bank2 col): [a^2 b*c_s + ab*c_{s+1} ; a^3 b*c_s + a^2 b*c_{s+1}]
    Boot tile (r=0, H_0=0): bank col = [0 ; b*c_1],
      bank2 col = [ab*c_1 ; a^2 b*c_1].
    """
    ab = a * b
    ce = Cc[:, 0::2, :].astype(np.float64)    # c_{2r}   [B, NPAIR, H]
    co = Cc[:, 1::2, :].astype(np.float64)    # c_{2r+1}
    out = np.empty((NPAIR, 2 * H, 2 * H), np.float32)
    # bank col
    out[:, :H, :H] = (b * ce).transpose(1, 2, 0)
    out[:, H:, :H] = (ab * ce + b * co).transpose(1, 2, 0)
    # bank2 col
    out[:, :H, H:] = (a * (ab * ce + b * co)).transpose(1, 2, 0)
    out[:, H:, H:] = (a * a * (ab * ce + b * co)).transpose(1, 2, 0)
    # boot overrides (c_0 unused, H_0 = 0)
    c1 = co[:, 0, :]                          # [B, H]
    out[0, :H, :H] = 0.0
    out[0, H:, :H] = (b * c1).T
    out[0, :H, H:] = (ab * c1).T
    out[0, H:, H:] = (a * ab * c1).T
    return out


def _build_program_pair():
    """Pair-corrected scheme v2: 2 timesteps per tanh round (S/2 rounds).

    PSUM bank halves = [H_s ; H_{s+1}^pred]; one bf16 ACT tanh covers both
    and feeds the (tiny) tanh-coupling matmuls LT1/LT2 in bf16; a second f32
    tanh feeds the f32 output matvec.  The c-injection is folded into the
    f32 decay matmul LH via a host-prescaled C'' tile DMA'd into the hm
    tile, whose lower half gets H_{s-1} added by one DVE op:
        hm = [b*c_{s+1} ; (b/a)*c_s + H_{s-1}]
        LH @ hm = [a*H_{s-1}+b*c_s ; a^2*H_{s-1}+ab*c_s+b*c_{s+1}]
    """
    nc = bacc.Bacc("TRN2", target_bir_lowering=False, debug=False)

    BF16 = mybir.dt.bfloat16
    GDT = BF16 if os.environ.get("LNN_GDT", "bf16") == "bf16" else F32

    in_C = nc.dram_tensor("in_C", (NPAIR, 2 * H, B), F32,
                          kind="ExternalInput").ap()
    ins = {}
    for nm in ("LH", "LB"):
        ins[nm] = nc.dram_tensor(f"in_{nm}", (2 * H, 2 * H), F32,
                                 kind="ExternalInput").ap()
    for nm in ("LT1", "LT2"):
        ins[nm] = nc.dram_tensor(f"in_{nm}", (2 * H, 2 * H), GDT,
                                 kind="ExternalInput").ap()
    ins["LO"] = nc.dram_tensor("in_LO", (2 * H, 2), F32,
                               kind="ExternalInput").ap()
    out_dram = nc.dram_tensor("out", (NSEGP, 2, SEGP * B), F32,
                              kind="ExternalOutput").ap()

    TANH = mybir.ActivationFunctionType.Tanh

    with tile.TileContext(nc) as tc:
        with (
            tc.tile_pool(name="wts", bufs=1) as wts,
            tc.tile_pool(name="thp", bufs=4) as thp,
            tc.tile_pool(name="thf", bufs=3) as thfp,
            tc.tile_pool(name="thz", bufs=1) as thz,
            tc.tile_pool(name="osb", bufs=2) as osbp,
            tc.tile_pool(name="hmp", bufs=8) as hmp,
            tc.tile_pool(name="hbank", bufs=4, space="PSUM") as hbank,
            tc.tile_pool(name="obank", bufs=3, space="PSUM") as obankp,
        ):
            t_w = {}
            for nm in ("LH", "LB"):
                t_w[nm] = wts.tile([2 * H, 2 * H], F32, name=f"t_{nm}")
                nc.sync.dma_start(out=t_w[nm], in_=ins[nm])
            for nm in ("LT1", "LT2"):
                t_w[nm] = wts.tile([2 * H, 2 * H], GDT, name=f"t_{nm}")
                nc.sync.dma_start(out=t_w[nm], in_=ins[nm])
            t_w["LO"] = wts.tile([2 * H, 2], F32, name="t_LO")
            nc.sync.dma_start(out=t_w["LO"], in_=ins["LO"])

            t_zero = thz.tile([2 * H, B], GDT, tag="t1zero")
            nc.vector.memset(t_zero, 0.0)
            t_osb = [osbp.tile([2, SEGP * B], F32, tag="osb", name=f"t_osb{i}")
                     for i in range(2)]

            # boot: bank_0 = [0 ; b*c_1]  (C''_0 half0 = b*c_1)
            t_hm = hmp.tile([2 * H, B], F32, tag="hm")
            nc.sync.dma_start(out=t_hm, in_=in_C[0])
            bank = hbank.tile([2 * H, B], F32, tag="bank")
            nc.tensor.matmul(bank, t_w["LB"], t_hm, start=True, stop=True)

            prev_bank = bank
            prev_T1 = t_zero
            prev_thf = None           # f32 tanh pair awaiting its out matvec
            pending = []              # [(ob_tile, slot_index)] not yet evac'd

            def flush_one():
                ob_t, m = pending.pop(0)
                seg, slot = divmod(m, SEGP)
                nc.vector.tensor_copy(
                    t_osb[seg % 2][0:2, slot * B:(slot + 1) * B], ob_t)
                if slot == SEGP - 1:
                    nc.sync.dma_start(out=out_dram[seg],
                                      in_=t_osb[seg % 2][0:2, :])

            for r in range(1, NPAIR):
                t_hm = hmp.tile([2 * H, B], F32, tag="hm")
                nc.sync.dma_start(out=t_hm, in_=in_C[r])

                bank = hbank.tile([2 * H, B], F32, tag="bank")
                # bf16 matmul first (FWL-friendly after last round's bf16 LT1)
                nc.tensor.matmul(bank, t_w["LT2"], prev_T1,
                                 start=True, stop=False)
                # the two f32 matmuls adjacent: previous round's out matvec,
                # then the decay+input injection
                if prev_thf is not None:
                    ob = obankp.tile([2, B], F32, tag="ob")
                    nc.tensor.matmul(ob, t_w["LO"], prev_thf,
                                     start=True, stop=True)
                    pending.append((ob, r - 2))

                # tanh pair: bf16 for the coupling path (critical), f32 for
                # the output matvec (off critical path)
                T1 = thp.tile([2 * H, B], GDT, tag="t1")
                nc.scalar.activation(T1, prev_bank, TANH)
                t_thf = thfp.tile([2 * H, B], F32, tag="thf")
                nc.scalar.activation(t_thf, prev_bank, TANH)

                # hm lower half += H_{s-1} (from prev bank)
                nc.vector.tensor_add(t_hm[H:, :], t_hm[H:, :],
                                     prev_bank[H:, :])

                if len(pending) > 1:
                    flush_one()

                nc.tensor.matmul(bank, t_w["LH"], t_hm, start=False,
                                 stop=False)
                nc.tensor.matmul(bank, t_w["LT1"], T1, start=False, stop=True)

                prev_bank, prev_T1, prev_thf = bank, T1, t_thf

            # tail: emit out matvecs for the last two tanh pairs, flush all
            ob = obankp.tile([2, B], F32, tag="ob")
            nc.tensor.matmul(ob, t_w["LO"], prev_thf, start=True, stop=True)
            pending.append((ob, NPAIR - 2))
            t_thf = thfp.tile([2 * H, B], F32, tag="thf")
            nc.scalar.activation(t_thf, prev_bank, TANH)
            ob = obankp.tile([2, B], F32, tag="ob")
            nc.tensor.matmul(ob, t_w["LO"], t_thf, start=True, stop=True)
            pending.append((ob, NPAIR - 1))
            while pending:
                flush_one()   # final segment's DMA fires on its last slot

    nc.compile()
    return nc



def _pair_weights(a, b, W_hh, W_out):
    """Host lhsT matrices for the pair-corrected scheme (f64 in)."""
    import ml_dtypes
    gdt = (ml_dtypes.bfloat16 if os.environ.get("LNN_GDT", "bf16") == "bf16"
           else np.float32)
    W = W_hh.astype(np.float64)
    wout = W_out[0].astype(np.float64)
    ab, a2, a2b = a * b, a * a, a * a * b

    def blk(v):
        return (v[:, None] * W).T

    LH = np.zeros((2 * H, 2 * H))
    LH[:H, H:] = np.eye(H)
    LH[H:, :H] = np.diag(a)
    LH[H:, H:] = np.diag(a2)
    LT1 = np.zeros((2 * H, 2 * H))
    LT1[:H, :H] = blk(-0.5 * b + 1.5 * ab)
    LT1[:H, H:] = blk(-0.5 * ab + 1.5 * a2b - 1.5 * b)
    LT1[H:, :H] = blk(1.5 * b)
    LT1[H:, H:] = blk(1.5 * ab + 2.5 * b)
    LT2 = np.zeros((2 * H, 2 * H))
    LT2[:H, :H] = blk(1.5 * ab)
    LT2[:H, H:] = blk(1.5 * a2b)
    LT2[H:, :H] = blk(-3.0 * ab)
    LT2[H:, H:] = blk(-3.0 * a2b)
    LB = np.zeros((2 * H, 2 * H))
    LB[:H, H:] = np.eye(H)
    LO = np.zeros((2 * H, 2))
    LO[:H, 0] = wout
    LO[H:, 1] = wout
    return {"in_LH": LH.astype(np.float32),
            "in_LB": LB.astype(np.float32),
            "in_LT1": LT1.astype(gdt),
            "in_LT2": LT2.astype(gdt),
            "in_LO": LO.astype(np.float32)}



def _host_precompute(x, W_in, b_in, W_hh, W_ih, bias, tau, W_out, b_out):
    x = np.asarray(x, dtype=np.float32)
    W_in = np.asarray(W_in, dtype=np.float32)
    b_in = np.asarray(b_in, dtype=np.float32)
    W_hh = np.asarray(W_hh, dtype=np.float32)
    W_ih = np.asarray(W_ih, dtype=np.float32)
    bias = np.asarray(bias, dtype=np.float32)
    tau = np.asarray(tau, dtype=np.float32)
    W_out = np.asarray(W_out, dtype=np.float32)

    W_comb = W_ih @ W_in                      # [H, BIN]
    b_comb = W_ih @ b_in + bias               # [H]
    C = x @ W_comb.T + b_comb                 # [B_FULL, S, H] f32

    t = np.linspace(0.0, 1.0, S).astype(np.float32)
    dt = np.float64(t[1]) - np.float64(t[0])
    d = 1.0 / tau.astype(np.float64)
    a = np.exp(-d * dt)
    b = 1.0 - a

    Wp = (1.5 * b[:, None] * W_hh.astype(np.float64)).T   # lhsT [k, j]
    Wm = (-0.5 * b[:, None] * W_hh.astype(np.float64)).T
    wout = W_out[0].astype(np.float64)                    # [H]

    Aev = np.zeros((2 * H, H + 1), np.float64)
    Aev[:H, :H] = Wp
    Aev[H:, :H] = Wm
    Aev[:H, H] = wout
    Aod = np.zeros((2 * H, H + 1), np.float64)
    Aod[:H, :H] = Wm
    Aod[H:, :H] = Wp
    Aod[H:, H] = wout
    # tail round index S (=1024, even): th_S lives in half S%2
    Atl = np.zeros((2 * H, H + 1), np.float64)
    if S % 2 == 0:
        Atl[:H, H] = wout
    else:
        Atl[H:, H] = wout
    Db = np.zeros((H, H + 1), np.float64)
    Db[:, :H] = np.diag(b)
    Da = np.diag(a)

    return C, {
        "in_Aev": Aev.astype(np.float32),
        "in_Aod": Aod.astype(np.float32),
        "in_Atl": Atl.astype(np.float32),
        "in_Db": Db.astype(np.float32),
        "in_Da": Da.astype(np.float32),
    }


def kernel(x, W_in, b_in, W_hh, W_ih, bias, tau, W_out, b_out):
    C, wmaps = _host_precompute(x, W_in, b_in, W_hh, W_ih, bias, tau,
                                W_out, b_out)
    b_out = np.asarray(b_out, dtype=np.float32)

    if SCHEME in ("pair", "pairz", "v7"):
        t = np.linspace(0.0, 1.0, S).astype(np.float32)
        dt = np.float64(t[1]) - np.float64(t[0])
        d = 1.0 / np.asarray(tau, dtype=np.float32).astype(np.float64)
        a = np.exp(-d * dt)
        b = 1.0 - a
        if SCHEME == "v7":
            wmaps = _v7_weights(a, b, np.asarray(W_hh, np.float32),
                                np.asarray(W_out, np.float32))
            a2v = wmaps["in_a2"].ravel()
            a2u = float(a2v[0]) if np.all(a2v == a2v[0]) else None
            builder = lambda: _build_program_v7(a2_imm=a2u)
        elif SCHEME == "pairz":
            wmaps = _pairz_weights(a, b, np.asarray(W_hh, np.float32),
                                   np.asarray(W_out, np.float32))
            builder = _build_program_pairz
        else:
            wmaps = _pair_weights(a, b, np.asarray(W_hh, np.float32),
                                  np.asarray(W_out, np.float32))
            builder = _build_program_pair
            # prescaled pair C'': tile r = [b*c_{2r+1} ; (b/a)*c_{2r}]
            bf = b.astype(np.float32)[None, :]
            baf = (b / a).astype(np.float32)[None, :]
    else:
        builder = _build_program

    if "nc" not in _cached:
        _cached["nc"] = builder()
    nc = _cached["nc"]

    in_maps = []
    for i in range(N_CORES):
        Cc = C[i * B:(i + 1) * B]                        # [B, S, H]
        if SCHEME == "v7":
            Cb, C2 = _v7_cc(Cc, a, b)
            in_maps.append({"in_Cb": Cb, "in_C2": C2, **wmaps})
            continue
        if SCHEME == "pairz":
            C_core = _pairz_cc(Cc, a, b)                 # [NPAIR, 2H, 2H]
        elif SCHEME == "pair":
            odd = (Cc[:, 1::2, :] * bf).transpose(1, 2, 0)   # [NPAIR, H, B]
            even = (Cc[:, 0::2, :] * baf).transpose(1, 2, 0)
            C_core = np.ascontiguousarray(
                np.concatenate([odd, even], axis=1))     # [NPAIR, 2H, B]
        else:
            C_core = np.ascontiguousarray(Cc.transpose(1, 2, 0))  # [S, H, B]
        in_maps.append({"in_C": C_core, **wmaps})

    core_ids = list(range(N_CORES))
    _cached["in_maps"] = in_maps
    res = run_bass_kernel_spmd(nc, in_maps, core_ids)

    out = np.empty((B_FULL, S, 1), dtype=np.float32)
    for i in range(N_CORES):
        if SCHEME == "v7":
            dev = res.results[i]["out"].reshape(NGRP, 2, GRP, B)
            out[i * B:(i + 1) * B, :, 0] = (
                dev.transpose(3, 0, 2, 1).reshape(B, S) + b_out[0])
            continue
        if SCHEME in ("pair", "pairz"):
            dev = res.results[i]["out"].reshape(NSEGP, 2, SEGP, B)
            dev = dev.transpose(0, 2, 1, 3).reshape(S, B)   # [o, b]
        else:
            dev = res.results[i]["out"].reshape(S, B)        # [s, b_local]
        out[i * B:(i + 1) * B, :, 0] = dev.T + b_out[0]
    return out


def _in_maps_for_test(C, wmaps):
    maps = []
    for i in range(N_CORES):
        C_core = np.ascontiguousarray(C[i * B:(i + 1) * B].transpose(1, 2, 0))
        maps.append({"in_C": C_core, **wmaps})
    return maps

